# revision 4
# baseline (speedup 1.0000x reference)
"""AlphaFold-style node update (row-gated-attn + col-gated-attn + FF) on 8 TRN2 cores.

Sharding: L (query rows) across 8 cores, weights replicated.  v2 pipeline:
  - pair streamed f32->bf16 via SWDGE cast-DMA in a grouped-k layout
    (partition p holds keys k=8p+j, j=0..7 -> 4KB contiguous lines).  The
    k-permutation is applied consistently to K/V (same grouped msa load), so
    attention (sum over keys) is unaffected.
  - per block (8 q rows x all 1024 k): DVE square (bf16 2x) + one batched
    reduce for sumsq; PE transposes [k,c]->[c,k] per (q,j); ACT copies
    PSUM->SBUF; per-j S-matmuls vs Wstat (proj + sum column); r = rsqrt chain;
    DVE assembly of the bias S_all[k, (j q h)] * r.
  - row attention / gate / residual processed in two 64-q halves interleaved
    with the pair loop; AllGather of x1^T split in two so its latency hides
    under the remaining pair blocks.
  - col attention + FF also per half; K/V col projections chunked per
    gathered half.
"""
import numpy as np

import concourse.bass as bass
import concourse.bacc as bacc
import concourse.tile as tile
from concourse import mybir
from concourse.bass_utils import run_bass_kernel_spmd
from concourse.masks import make_identity

F32 = mybir.dt.float32
BF16 = mybir.dt.bfloat16
AX = mybir.AxisListType
OP = mybir.AluOpType
AF = mybir.ActivationFunctionType

NCORES = 8
L = 1024          # sequence length
D = 256           # d_msa
C = 128           # d_pair
H = 8             # heads
DH = 32           # head dim
MYQ = L // NCORES  # 128 q rows per core
T = L // 128      # 8 k-tiles (tile j holds keys k = 8p + j)
SCALE = 1.0 / float(np.sqrt(DH))
EPS = 1e-5
QBLK = 8          # q rows per pair-loop block
NBLK = MYQ // QBLK  # 16 blocks
HALF = MYQ // 2   # 64 q rows per attention half


def build():
    nc = bacc.Bacc("TRN2", target_bir_lowering=False, debug=False, num_devices=NCORES)

    def inp(name, shape):
        return nc.dram_tensor(name, shape, F32, kind="ExternalInput").ap()

    msa = inp("msa", [L, D])              # full msa (replicated)
    msa_my = inp("msa_my", [MYQ, D])      # this core's q rows
    pair_my = inp("pair_my", [MYQ, L, C])  # this core's pair slice
    ln_node_g = inp("ln_node_g", [1, D])
    ln_node_b = inp("ln_node_b", [1, D])
    ln_pair_g = inp("ln_pair_g", [C, 1])
    ln_pair_b = inp("ln_pair_b", [C, 1])
    row_Wq = inp("row_Wq", [D, D])
    row_Wk = inp("row_Wk", [D, D])
    row_Wv = inp("row_Wv", [D, D])
    row_Wb = inp("row_Wb", [C, H])
    row_Wg = inp("row_Wg", [D, D])
    row_bg = inp("row_bg", [1, D])
    row_Wo = inp("row_Wo", [D, D])
    row_bo = inp("row_bo", [1, D])
    col_Wq = inp("col_Wq", [D, D])
    col_Wk = inp("col_Wk", [D, D])
    col_Wv = inp("col_Wv", [D, D])
    col_Wg = inp("col_Wg", [D, D])
    col_bg = inp("col_bg", [1, D])
    col_Wo = inp("col_Wo", [D, D])
    col_bo = inp("col_bo", [1, D])
    ff_ln_g = inp("ff_ln_g", [1, D])
    ff_ln_b = inp("ff_ln_b", [1, D])
    ff_W1 = inp("ff_W1", [D, D])
    ff_b1 = inp("ff_b1", [D, 1])
    ff_W2 = inp("ff_W2", [D, D])
    ff_b2 = inp("ff_b2", [1, D])

    out_my = nc.dram_tensor("out_my", [MYQ, D], F32, kind="ExternalOutput").ap()

    # collective bounce buffers per half (x1^T, bf16): core gives [D, HALF]
    gin = [nc.dram_tensor(f"gather_in{h}", [D, HALF], BF16) for h in range(2)]
    gout = [nc.dram_tensor(f"gather_out{h}", [NCORES * D, HALF], BF16,
                           addr_space="Shared") for h in range(2)]

    import os
    reps = int(os.environ.get("KREPS", "1"))
    with tile.TileContext(nc) as tc:
        for _ in range(reps):
            _body(nc, tc, locals())
    nc.compile()
    return nc


def _bcast(nc, pool, src_1xD, n_free=D, tag=None):
    """[128, n_free] tile = src row broadcast across partitions (DMA step-0)."""
    t = pool.tile([128, n_free], F32, tag=tag, name=tag)
    src = bass.AP(tensor=src_1xD.tensor, offset=src_1xD.offset,
                  ap=[[0, 128], src_1xD.ap[-1]])
    nc.sync.dma_start(out=t, in_=src)
    return t


def _body(nc, tc, v):
    msa, msa_my, pair_my = v["msa"], v["msa_my"], v["pair_my"]
    out_my, gin, gout = v["out_my"], v["gin"], v["gout"]

    from contextlib import ExitStack
    ctx = ExitStack()
    pers = ctx.enter_context(tc.tile_pool(name="pers", bufs=1))
    roll = ctx.enter_context(tc.tile_pool(name="roll", bufs=2))
    nat_pool = ctx.enter_context(tc.tile_pool(name="nat", bufs=3))
    sq_pool = ctx.enter_context(tc.tile_pool(name="sq", bufs=1))
    pT_pool = ctx.enter_context(tc.tile_pool(name="pT", bufs=3))
    st_pool = ctx.enter_context(tc.tile_pool(name="st", bufs=2))
    roll3 = ctx.enter_context(tc.tile_pool(name="roll3", bufs=3))
    pp_tp = ctx.enter_context(tc.tile_pool(name="pp_tp", bufs=2, space="PSUM"))
    pp_s = ctx.enter_context(tc.tile_pool(name="pp_s", bufs=3, space="PSUM"))
    pp_l = ctx.enter_context(tc.tile_pool(name="pp_l", bufs=2, space="PSUM"))

    def P(shape, dt=F32, tag=None):
        return pers.tile(shape, dt, tag=tag, name=tag)

    # ================= pair stream: issue first blocks ASAP =================
    # grouped-k: partition p <- keys 8p+j ; per-partition line = 4KB contiguous
    pair_g = pair_my.rearrange("q (p j) c -> p q j c", p=128)
    p_nats = []

    def issue_pair_dma(b):
        t = nat_pool.tile([128, QBLK, T, C], BF16, tag="p_nat", name="p_nat")
        nc.gpsimd.dma_start(out=t, in_=pair_g[:, b * QBLK:(b + 1) * QBLK, :, :])
        p_nats.append(t)

    for b in range(3):
        issue_pair_dma(b)

    # ================= constants =================
    ident = P([128, 128], BF16, tag="ident")
    make_identity(nc, ident)
    ones_col = P([1, 128], F32, tag="ones_col")     # rank-1 lhsT (K=1)
    nc.vector.memset(ones_col, 1.0)
    ones128 = P([128, 1], F32, tag="ones128")       # column of ones (K=128)
    nc.vector.memset(ones128, 1.0)
    ones_k = P([128, 1], BF16, tag="ones_k")        # denominator rhs
    nc.vector.memset(ones_k, 1.0)
    eps_t = P([128, 1], F32, tag="eps_t")
    nc.vector.memset(eps_t, (C ** 2) * EPS)         # C^2*eps for v128
    eps_s = P([128, 1], F32, tag="eps_s")
    nc.vector.memset(eps_s, EPS)

    # ================= weights: HWDGE f32 loads + ACT cast to bf16 =========
    def wbf(name):
        w = v[name]
        stage = roll.tile([128, 2, D], F32, tag="wstage", name="wstage")
        nc.sync.dma_start(out=stage, in_=w.rearrange("(a p) d -> p a d", p=128))
        tl = P([128, 2, D], BF16, tag=f"w_{name}")
        nc.scalar.copy(tl, stage)
        return tl

    rWq, rWk, rWv, rWg, rWo = map(wbf, ["row_Wq", "row_Wk", "row_Wv", "row_Wg", "row_Wo"])
    cWq, cWk, cWv, cWg, cWo = map(wbf, ["col_Wq", "col_Wk", "col_Wv", "col_Wg", "col_Wo"])
    fW1, fW2 = map(wbf, ["ff_W1", "ff_W2"])

    bias_rows = {}
    for name in ["row_bg", "row_bo", "col_bg", "col_bo", "ff_b2"]:
        t = P([1, D], F32, tag=f"b_{name}")
        nc.sync.dma_start(out=t, in_=v[name])
        bias_rows[name] = t
    b1T = P([128, 2], F32, tag="b1T")               # ff_b1 per-partition
    nc.sync.dma_start(out=b1T, in_=v["ff_b1"].rearrange("(a p) o -> p (a o)", p=128))

    G_node = _bcast(nc, pers, v["ln_node_g"], tag="G_node")
    B_node = _bcast(nc, pers, v["ln_node_b"], tag="B_node")
    G_ff = _bcast(nc, pers, v["ff_ln_g"], tag="G_ff")
    B_ff = _bcast(nc, pers, v["ff_ln_b"], tag="B_ff")

    # ================= pair-bias weights =================
    # Wstat[:,0:H] = C*(g.*Wb) - 1 (x) u,  u = sum_c g.*Wb ; col H = ones
    Wb_sb = P([C, H], F32, tag="Wb_sb")
    nc.sync.dma_start(out=Wb_sb, in_=v["row_Wb"])
    g_pair = P([C, 1], F32, tag="g_pair")
    nc.sync.dma_start(out=g_pair, in_=v["ln_pair_g"])
    b_pair = P([C, 1], F32, tag="b_pair")
    nc.sync.dma_start(out=b_pair, in_=v["ln_pair_b"])

    Wgb = P([C, H], F32, tag="Wgb")
    nc.vector.tensor_scalar_mul(Wgb, Wb_sb, g_pair)

    ps_small = pp_l.tile([128, 128], F32, tag="ps_lg", name="ps_small")
    nc.tensor.matmul(ps_small[0:1, 0:H], ones128, Wgb, start=True, stop=True)
    u_row = P([1, H], F32, tag="u_row")
    nc.scalar.mul(u_row, ps_small[0:1, 0:H], -1.0)  # -(u)
    ps_small2 = pp_l.tile([128, 128], F32, tag="ps_lg", name="ps_small")
    nc.tensor.matmul(ps_small2[0:1, 0:H], b_pair, Wb_sb, start=True, stop=True)
    w_row = P([1, H], F32, tag="w_row")
    nc.scalar.copy(w_row, ps_small2[0:1, 0:H])
    ps_u = pp_l.tile([128, 128], F32, tag="ps_lg", name="ps_small")
    nc.tensor.matmul(ps_u[:, 0:H], ones_col, u_row, start=True, stop=True)
    Wgb_s = P([C, H], F32, tag="Wgb_s")
    nc.vector.tensor_scalar_mul(Wgb_s, Wgb, float(C))
    Wstat = P([C, H + 1], BF16, tag="Wstat")
    nc.vector.tensor_add(Wstat[:, 0:H], ps_u[:, 0:H], Wgb_s)
    nc.vector.tensor_copy(Wstat[:, H:H + 1], ones_k)
    ps_w = pp_l.tile([128, 128], F32, tag="ps_lg", name="ps_small")
    nc.tensor.matmul(ps_w[:, 0:H], ones_col, w_row, start=True, stop=True)
    w_tile = P([128, H], F32, tag="w_tile")
    nc.scalar.copy(w_tile, ps_w[:, 0:H])

    # ================= x0 = LN(msa) (grouped rows), K/V/Q projections =======
    def layer_norm(dst_f32, dst_bf, xt, g_t, b_t, pool, n=128):
        """LN over free dim D for [n, D] tile in SBUF; writes f32 + bf16."""
        st = pool.tile([128, 6], F32, tag="ln_st", name="ln_st")
        nc.vector.bn_stats(st[0:n, :], xt)
        mv = pool.tile([128, 2], F32, tag="ln_mv", name="ln_mv")
        nc.vector.bn_aggr(mv[0:n, :], st[0:n, :])
        sq = pool.tile([128, 1], F32, tag="ln_sq", name="ln_sq")
        nc.scalar.activation(sq[0:n, :], mv[0:n, 1:2], AF.Sqrt, bias=eps_s[0:n, :], scale=1.0)
        r = pool.tile([128, 1], F32, tag="ln_r", name="ln_r")
        nc.vector.reciprocal(r[0:n, :], sq[0:n, :])
        mr = pool.tile([128, 1], F32, tag="ln_mr", name="ln_mr")
        nc.vector.tensor_mul(mr[0:n, :], mv[0:n, 0:1], r[0:n, :])
        nmr = pool.tile([128, 1], F32, tag="ln_nmr", name="ln_nmr")
        nc.vector.tensor_scalar_mul(nmr[0:n, :], mr[0:n, :], -1.0)
        xn = pool.tile([128, D], F32, tag="ln_xn", name="ln_xn")
        nc.scalar.activation(xn[0:n, :], xt, AF.Identity, bias=nmr[0:n, :], scale=r[0:n, :])
        nc.vector.tensor_mul(dst_f32, xn[0:n, :], g_t[0:n, :])
        nc.vector.tensor_add(dst_f32, dst_f32, b_t[0:n, :])
        nc.vector.tensor_copy(dst_bf, dst_f32)

    # full msa in grouped-k layout: row (j, p) = msa[8p+j]
    msa_g_t = roll.tile([128, T, D], F32, tag="msa_g", name="msa_g", bufs=1)
    nc.sync.dma_start(out=msa_g_t, in_=msa.rearrange("(p j) d -> p j d", p=128))
    x0_f = P([128, T, D], F32, tag="x0_f")
    x0_bf = P([128, T, D], BF16, tag="x0_bf")
    for j in range(T):
        layer_norm(x0_f[:, j, :], x0_bf[:, j, :], msa_g_t[:, j, :], G_node, B_node, roll)
    # my q rows (identity order)
    msa_my_t = roll.tile([128, D], F32, tag="msa_my", name="msa_my")
    nc.sync.dma_start(out=msa_my_t, in_=msa_my)
    x0my_f = P([128, D], F32, tag="x0my_f")
    x0my_bf = P([128, D], BF16, tag="x0my_bf")
    layer_norm(x0my_f, x0my_bf, msa_my_t, G_node, B_node, roll)

    def transpose_to(dst_bf, src_tiles, n, ncol=128):
        """src: n [ncol,128] bf16 APs -> dst [128, n*ncol] bf16 via PE+ACT."""
        ps = pp_tp.tile([128, T * 128], BF16, tag="tp", name="tp")
        for i in range(n):
            nc.tensor.transpose(ps[:, i * ncol:(i + 1) * ncol], src_tiles[i],
                                ident[0:ncol, 0:ncol])
        nc.scalar.copy(dst_bf[:, 0:n * ncol], ps[:, 0:n * ncol])

    x0T = P([128, 2, L], BF16, tag="x0T")
    for jm in range(2):
        transpose_to(x0T[:, jm, :], [x0_bf[:, j, jm * 128:(jm + 1) * 128] for j in range(T)], T)
    x0Tmy = P([128, 2, 128], BF16, tag="x0Tmy")
    for jm in range(2):
        transpose_to(x0Tmy[:, jm, :], [x0my_bf[:, jm * 128:(jm + 1) * 128]], 1)

    def project_T(dst, W_bf, xT_full, n_l, scale=None, c0=0):
        """dst cols [c0,c0+n_l): (x @ W)^T from xT_full cols [c0,c0+n_l)."""
        for jm in range(2):
            for q4 in range(0, n_l, 256):
                w = min(256, n_l - q4)
                ps = pp_s.tile([128, 288], F32, tag="proj", name="proj")
                for Dj in range(2):
                    nc.tensor.matmul(
                        ps[:, 0:w],
                        W_bf[:, Dj, jm * 128:(jm + 1) * 128],
                        xT_full[:, Dj, c0 + q4:c0 + q4 + w],
                        start=(Dj == 0), stop=(Dj == 1))
                if scale is None:
                    nc.scalar.copy(dst[:, jm, c0 + q4:c0 + q4 + w], ps[:, 0:w])
                else:
                    nc.scalar.mul(dst[:, jm, c0 + q4:c0 + q4 + w], ps[:, 0:w], scale)

    def project_V(dst, W_bf, xT_full, tiles):
        """dst [128, t, D] bf16 = x @ W for the given k-tiles."""
        for t in tiles:
            for dh in range(0, D, 256):
                ps = pp_s.tile([128, 288], F32, tag="proj", name="proj")
                for Dj in range(2):
                    nc.tensor.matmul(
                        ps[:, 0:256],
                        xT_full[:, Dj, t * 128:(t + 1) * 128],
                        W_bf[:, Dj, dh:dh + 256],
                        start=(Dj == 0), stop=(Dj == 1))
                nc.scalar.copy(dst[:, t, dh:dh + 256], ps[:, 0:256])

    KT_row = P([128, 2, L], BF16, tag="KT_row")
    project_T(KT_row, rWk, x0T, L)
    QT_row = P([128, 2, 128], BF16, tag="QT_row")
    project_T(QT_row, rWq, x0Tmy, 128, scale=SCALE)
    V_row = P([128, T, D], BF16, tag="V_row")
    project_V(V_row, rWv, x0T, range(T))

    # ================= pair loop =================
    S_all = P([128, T * MYQ * H], BF16, tag="S_all")  # free = j*1024 + q*8 + h

    def pair_block(b):
        p_nat = p_nats[b]
        flat = p_nat.rearrange("p q j c -> p (q j c)")
        p_sq = sq_pool.tile([128, QBLK * T * C], BF16, tag="p_sq", name="p_sq")
        nc.vector.tensor_mul(p_sq, flat, flat)          # bf16 2x
        sumsq = st_pool.tile([128, QBLK * T], F32, tag="sumsq", name="sumsq")
        nc.vector.tensor_reduce(
            out=sumsq, in_=p_sq.rearrange("p (qj c) -> p qj c", c=C),
            axis=AX.X, op=OP.add)

        sums = st_pool.tile([128, QBLK * T], F32, tag="sums", name="sums")
        ps_S_list = []
        for hb in range(2):
            ps_S = pp_s.tile([128, 288], F32, tag="proj", name="ps_S")
            ps_S_list.append(ps_S)
            for qi in range(4):
                qq = hb * 4 + qi
                ps_t = pp_tp.tile([128, T * 128], BF16, tag="tp", name="tp")
                for j in range(T):
                    nc.tensor.transpose(ps_t[:, j * 128:(j + 1) * 128],
                                        p_nat[:, qq, j, :], ident)
                pT = pT_pool.tile([128, T * 128], BF16, tag="pT", name="pT")
                nc.scalar.copy(pT, ps_t)
                for j in range(T):
                    nc.tensor.matmul(
                        ps_S[:, (qi * T + j) * 9:(qi * T + j) * 9 + 9],
                        pT[:, j * 128:(j + 1) * 128], Wstat,
                        start=True, stop=True)
            nc.vector.tensor_copy(
                sums[:, hb * 32:(hb + 1) * 32],
                bass.AP(tensor=ps_S.tensor, offset=ps_S.offset + 8,
                        ap=[ps_S.ap[0], [9, 32]]))

        # r = 1 / sqrt(C*sumsq - sum^2 + C^2 eps)
        t1 = st_pool.tile([128, QBLK * T], F32, tag="t1", name="t1")
        nc.vector.tensor_mul(t1, sums, sums)
        v128 = st_pool.tile([128, QBLK * T], F32, tag="v128", name="v128")
        nc.vector.tensor_scalar_mul(v128, sumsq, float(C))
        nc.vector.tensor_sub(v128, v128, t1)
        sqv = st_pool.tile([128, QBLK * T], F32, tag="sqv", name="sqv")
        nc.scalar.activation(sqv, v128, AF.Sqrt, bias=eps_t, scale=1.0)
        r_all = st_pool.tile([128, QBLK * T], F32, tag="r_all", name="r_all")
        nc.vector.reciprocal(r_all, sqv)

        # S_all[p, j*1024 + q*8 + h] = ps_S[(qi*T+j)*9 + h] * r[qq*T+j]
        for hb in range(2):
            ps_S = ps_S_list[hb]
            out_ap = bass.AP(
                tensor=S_all.tensor,
                offset=S_all.offset + (b * QBLK + hb * 4) * H,
                ap=[S_all.ap[0], [H, 4], [MYQ * H, T], [1, H]])
            in_ap = bass.AP(
                tensor=ps_S.tensor, offset=ps_S.offset,
                ap=[ps_S.ap[0], [9 * T, 4], [9, T], [1, H]])
            r_ap = bass.AP(
                tensor=r_all.tensor, offset=r_all.offset + hb * 4 * T,
                ap=[r_all.ap[0], [T, 4], [1, T], [0, H]])
            nc.vector.tensor_tensor(out=out_ap, in0=in_ap, in1=r_ap, op=OP.mult)

    # ================= attention / gate / residual per 64-q half ===========
    def attention_half(h0, KT, QT, V, S_bias, w_t, o_half):
        """k-part attention for q rows [h0, h0+HALF); o_half [64, D] bf16."""
        for h8 in range(H):
            ps_o = pp_l.tile([128, 128], F32, tag="ps_o", name="ps_o", bufs=1)
            E = roll3.tile([128, T * HALF], BF16, tag="E", name="E")
            for j in range(T):
                ps_lg = pp_l.tile([128, 128], F32, tag="ps_lg", name="ps_lg")
                jh, rh = h8 // 4, (h8 % 4) * 32
                nc.tensor.matmul(
                    ps_lg[:, 0:HALF],
                    KT[rh:rh + 32, jh, j * 128:(j + 1) * 128],
                    QT[rh:rh + 32, jh, h0:h0 + HALF],
                    start=True, stop=(S_bias is None),
                    tile_position=(rh, 0))
                if S_bias is not None:
                    bias_ap = bass.AP(
                        tensor=S_bias.tensor,
                        offset=S_bias.offset + j * MYQ * H + h0 * H + h8,
                        ap=[S_bias.ap[0], [H, HALF]])
                    nc.tensor.matmul(ps_lg[:, 0:HALF], ident, bias_ap,
                                     start=False, stop=True)
                if w_t is not None:
                    nc.scalar.activation(E[:, j * HALF:(j + 1) * HALF],
                                         ps_lg[:, 0:HALF],
                                         AF.Exp, bias=w_t[:, h8:h8 + 1], scale=1.0)
                else:
                    nc.scalar.activation(E[:, j * HALF:(j + 1) * HALF],
                                         ps_lg[:, 0:HALF],
                                         AF.Exp, bias=0.0, scale=1.0)
            for j in range(T):
                nc.tensor.matmul(ps_o[0:HALF, 0:DH], E[:, j * HALF:(j + 1) * HALF],
                                 V[:, j, h8 * DH:(h8 + 1) * DH],
                                 start=(j == 0), stop=False)
                nc.tensor.matmul(ps_o[0:HALF, DH:DH + 1], E[:, j * HALF:(j + 1) * HALF],
                                 ones_k, start=(j == 0), stop=(j == T - 1))
            recip = roll3.tile([128, 1], F32, tag="recip", name="recip")
            nc.vector.reciprocal(recip[0:HALF, :], ps_o[0:HALF, DH:DH + 1])
            nc.vector.tensor_scalar_mul(o_half[0:HALF, h8 * DH:(h8 + 1) * DH],
                                        ps_o[0:HALF, 0:DH], recip[0:HALF, :])

    def gate_proj_residual_half(h0, xT_my, Wg_bf, bg_row, Wo_bf, bo_row, o_half,
                                x_prev_half, x_new_f, x_new_bf, hpool):
        """x_new[0:64] = x_prev[0:64] + (sigmoid(x@Wg+bg) * o) @ Wo + bo."""
        ps_g = pp_s.tile([128, 288], F32, tag="proj", name="proj")
        for Dj in range(2):
            nc.tensor.matmul(ps_g[0:HALF, 0:256], xT_my[:, Dj, h0:h0 + HALF],
                             Wg_bf[:, Dj, :], start=(Dj == 0), stop=False)
        nc.tensor.matmul(ps_g[0:HALF, 0:256], ones_col[:, 0:HALF], bg_row,
                         start=False, stop=True)
        g_sb = hpool.tile([128, D], BF16, tag="g_sb", name="g_sb")
        nc.scalar.activation(g_sb[0:HALF, :], ps_g[0:HALF, 0:256], AF.Sigmoid,
                             bias=0.0, scale=1.0)
        go = hpool.tile([128, D], BF16, tag="go", name="go")
        nc.vector.tensor_mul(go[0:HALF, :], g_sb[0:HALF, :], o_half[0:HALF, :])
        goT = hpool.tile([128, 2, HALF], BF16, tag="goT", name="goT")
        for jm in range(2):
            transpose_to(goT[:, jm, :], [go[0:HALF, jm * 128:(jm + 1) * 128]], 1,
                         ncol=HALF)
        ps_y = pp_s.tile([128, 288], F32, tag="proj", name="proj")
        for Dj in range(2):
            nc.tensor.matmul(ps_y[0:HALF, 0:256], goT[:, Dj, 0:HALF],
                             Wo_bf[:, Dj, :], start=(Dj == 0), stop=False)
        nc.tensor.matmul(ps_y[0:HALF, 0:256], ones_col[:, 0:HALF], bo_row,
                         start=False, stop=True)
        nc.vector.tensor_add(x_new_f[0:HALF, :], x_prev_half[0:HALF, :],
                             ps_y[0:HALF, 0:256])
        nc.vector.tensor_copy(x_new_bf[0:HALF, :], x_new_f[0:HALF, :])

    # storage for halves
    x1_f = [P([128, D], F32, tag=f"x1_f{h}") for h in range(2)]
    x1_bf = [P([128, D], BF16, tag=f"x1_bf{h}") for h in range(2)]
    o_row = [P([128, D], BF16, tag=f"o_row{h}") for h in range(2)]
    x1Tmy = P([128, 2, 128], BF16, tag="x1Tmy")
    x1T = P([128, 2, L], BF16, tag="x1T")
    KT_col = P([128, 2, L], BF16, tag="KT_col")
    V_col = P([128, T, D], BF16, tag="V_col")

    def row_half(h):
        h0 = h * HALF
        attention_half(h0, KT_row, QT_row, V_row, S_all, w_tile, o_row[h])
        gate_proj_residual_half(h0, x0Tmy, rWg, bias_rows["row_bg"], rWo,
                                bias_rows["row_bo"], o_row[h],
                                x0my_f[h0:h0 + HALF, :], x1_f[h], x1_bf[h], roll3)
        # transpose x1 half into x1Tmy cols [h0, h0+HALF)
        for jm in range(2):
            transpose_to(x1Tmy[:, jm, h0:h0 + HALF],
                         [x1_bf[h][0:HALF, jm * 128:(jm + 1) * 128]], 1, ncol=HALF)
        # all-gather this half of x1^T
        for jm in range(2):
            nc.sync.dma_start(out=gin[h].ap()[jm * 128:(jm + 1) * 128, :],
                              in_=x1Tmy[:, jm, h0:h0 + HALF])
        nc.gpsimd.collective_compute(
            "AllGather", OP.bypass,
            replica_groups=[list(range(NCORES))],
            ins=[gin[h].ap().opt()],
            outs=[gout[h].ap().opt()])
        # x1T cols [h*512 + i*64 + q'] = core i's half h
        gout_r = gout[h].ap().rearrange("(i a p) q -> p a i q", i=NCORES, a=2)
        x1T_5d = x1T.rearrange("p a (hh i q) -> p a hh i q", hh=2, i=NCORES)
        for jm in range(2):
            nc.sync.dma_start(out=x1T_5d[:, jm, h, :, :], in_=gout_r[:, jm, :, :])
        # col K/V projections for this half's tiles (t = 4h .. 4h+3)
        project_T(KT_col, cWk, x1T, 512, c0=h * 512)
        project_V(V_col, cWv, x1T, range(4 * h, 4 * h + 4))

    # ---- pair loop with interleaved row-attention halves ----
    for b in range(NBLK):
        if b + 3 < NBLK:
            issue_pair_dma(b + 3)
        pair_block(b)
        if b == NBLK // 2 - 1:
            row_half(0)
    row_half(1)

    # ---- col attention (per half) + FF ----
    QT_col = P([128, 2, 128], BF16, tag="QT_col")
    project_T(QT_col, cWq, x1Tmy, 128, scale=SCALE)

    for h in range(2):
        h0 = h * HALF
        o_col = roll3.tile([128, D], BF16, tag="o_col", name="o_col")
        attention_half(h0, KT_col, QT_col, V_col, None, None, o_col)
        x2_f = roll3.tile([128, D], F32, tag="x2_f", name="x2_f")
        x2_bf = roll3.tile([128, D], BF16, tag="x2_bf", name="x2_bf")
        gate_proj_residual_half(h0, x1Tmy, cWg, bias_rows["col_bg"], cWo,
                                bias_rows["col_bo"], o_col,
                                x1_f[h], x2_f, x2_bf, roll3)

        # FF on this half
        h_f = roll3.tile([128, D], F32, tag="h_f", name="h_f")
        h_bf = roll3.tile([128, D], BF16, tag="h_bf", name="h_bf")
        layer_norm(h_f[0:HALF, :], h_bf[0:HALF, :], x2_f[0:HALF, :],
                   G_ff, B_ff, roll3, n=HALF)
        hT = roll3.tile([128, 2, HALF], BF16, tag="hT", name="hT")
        for jm in range(2):
            transpose_to(hT[:, jm, :], [h_bf[0:HALF, jm * 128:(jm + 1) * 128]], 1,
                         ncol=HALF)
        a1T = roll3.tile([128, 2, HALF], BF16, tag="a1T", name="a1T")
        for jm in range(2):
            ps_z = pp_s.tile([128, 288], F32, tag="proj", name="proj")
            for Dj in range(2):
                nc.tensor.matmul(ps_z[:, 0:HALF], fW1[:, Dj, jm * 128:(jm + 1) * 128],
                                 hT[:, Dj, :], start=(Dj == 0), stop=(Dj == 1))
            nc.scalar.activation(a1T[:, jm, :], ps_z[:, 0:HALF], AF.Relu,
                                 bias=b1T[:, jm:jm + 1], scale=1.0)
        ps_y = pp_s.tile([128, 288], F32, tag="proj", name="proj")
        for jm in range(2):
            nc.tensor.matmul(ps_y[0:HALF, 0:256], a1T[:, jm, :], fW2[:, jm, :],
                             start=(jm == 0), stop=False)
        nc.tensor.matmul(ps_y[0:HALF, 0:256], ones_col[:, 0:HALF],
                         bias_rows["ff_b2"], start=False, stop=True)
        out_sb = roll3.tile([128, D], F32, tag="out_sb", name="out_sb")
        nc.vector.tensor_add(out_sb[0:HALF, :], x2_f[0:HALF, :], ps_y[0:HALF, 0:256])
        nc.sync.dma_start(out=out_my[h0:h0 + HALF, :], in_=out_sb[0:HALF, :])
    ctx.close()


_NC_CACHE = None


def make_in_maps(common, msa, pair):
    in_maps = []
    for i in range(NCORES):
        m = dict(common)
        m["msa_my"] = np.ascontiguousarray(msa[i * MYQ:(i + 1) * MYQ, :])
        m["pair_my"] = np.ascontiguousarray(pair[i * MYQ:(i + 1) * MYQ, :, :])
        in_maps.append(m)
    return in_maps


def kernel(**inputs):
    global _NC_CACHE
    if _NC_CACHE is None:
        _NC_CACHE = build()
    nc = _NC_CACHE

    msa = np.asarray(inputs["msa"]).reshape(L, D).astype(np.float32)
    pair = np.asarray(inputs["pair"]).reshape(L, L, C).astype(np.float32)

    def f(name, shape):
        return np.ascontiguousarray(
            np.asarray(inputs[name]).reshape(shape).astype(np.float32))

    common = {
        "msa": msa,
        "ln_node_g": f("ln_node_g", (1, D)), "ln_node_b": f("ln_node_b", (1, D)),
        "ln_pair_g": f("ln_pair_g", (C, 1)), "ln_pair_b": f("ln_pair_b", (C, 1)),
        "row_Wq": f("row_Wq", (D, D)), "row_Wk": f("row_Wk", (D, D)),
        "row_Wv": f("row_Wv", (D, D)), "row_Wb": f("row_Wb", (C, H)),
        "row_Wg": f("row_Wg", (D, D)), "row_bg": f("row_bg", (1, D)),
        "row_Wo": f("row_Wo", (D, D)), "row_bo": f("row_bo", (1, D)),
        "col_Wq": f("col_Wq", (D, D)), "col_Wk": f("col_Wk", (D, D)),
        "col_Wv": f("col_Wv", (D, D)),
        "col_Wg": f("col_Wg", (D, D)), "col_bg": f("col_bg", (1, D)),
        "col_Wo": f("col_Wo", (D, D)), "col_bo": f("col_bo", (1, D)),
        "ff_ln_g": f("ff_ln_g", (1, D)), "ff_ln_b": f("ff_ln_b", (1, D)),
        "ff_W1": f("ff_W1", (D, D)), "ff_b1": f("ff_b1", (D, 1)),
        "ff_W2": f("ff_W2", (D, D)), "ff_b2": f("ff_b2", (1, D)),
    }
    in_maps = make_in_maps(common, msa, pair)
    res = run_bass_kernel_spmd(nc, in_maps, core_ids=list(range(NCORES)))
    out = np.concatenate([res.results[i]["out_my"] for i in range(NCORES)], axis=0)
    return out.reshape(1, L, D).astype(np.float32)


if __name__ == "__main__":
    build()
    print("build OK")


# revision 10
# speedup vs baseline: 1.4187x; 1.4187x over previous
"""AlphaFold-style node update (row-gated-attn + col-gated-attn + FF) on 8 TRN2 cores.

Sharding: L (query rows) across 8 cores, weights replicated.  v3 pipeline:
  - pair streamed f32->bf16 via SWDGE cast-DMA in a grouped-k layout
    (partition p holds keys k=8p+j -> 4KB contiguous lines); the permutation
    is applied consistently to K/V via the same grouped msa load, and
    attention is permutation-invariant over keys.
  - per block (8 q x 1024 k): fused custom DVE op t=a^2+b^2 (halves the
    square+reduce stream), add-fold, batched reduce -> sumsq; PE transposes
    [k,c]->[c,k]; ACT PSUM->SBUF copies; per-j S-matmuls vs Wstat (pair-bias
    projection + row-sum column); r = rsqrt chain; DVE assembly of
    S_half[k, (j q h)] * r.
  - setup (LN/x0T/K/V/Q projections, weight casts) interleaved into the
    first pair blocks; weights loaded f32 on HWDGE + ACT cast so gpsimd
    serves only the pair stream.
  - row attention / gate / residual per 64-q half: heads spread across pair
    blocks 8..11, AllGather of x1^T per half (first AG hidden under the
    remaining pair blocks); col attention + FF per half at the tail.
"""
import re
import numpy as np

import concourse.bass as bass
import concourse.bacc as bacc
import concourse.tile as tile
from concourse import mybir
from concourse import dve_ops
from concourse.dve_ops import DveOp
from concourse.dve_spec import Spec, Src0, Src1
from concourse.bass_utils import run_bass_kernel_spmd
from concourse.masks import make_identity

F32 = mybir.dt.float32
BF16 = mybir.dt.bfloat16
AX = mybir.AxisListType
OP = mybir.AluOpType
AF = mybir.ActivationFunctionType

NCORES = 8
L = 1024          # sequence length
D = 256           # d_msa
C = 128           # d_pair
H = 8             # heads
DH = 32           # head dim
MYQ = L // NCORES  # 128 q rows per core
T = L // 128      # 8 k-tiles (tile j holds keys k = 8p + j)
SCALE = 1.0 / float(np.sqrt(DH))
EPS = 1e-5
QBLK = 8          # q rows per pair-loop block
NBLK = MYQ // QBLK  # 16 blocks
HALF = MYQ // 2   # 64 q rows per attention half


def _make_sq2():
    """Custom DVE op: out = Src0^2 + Src1^2 (fused square + pairwise fold)."""
    for op in dve_ops.OPS:
        if op.name == "SQ2_ADD_ANT":
            return op
    op = DveOp(
        "SQ2_ADD_ANT",
        Spec(
            body=Src0 * Src0 + Src1 * Src1,
            reference=lambda in0, in1, s0, s1, imm2: (
                in0.astype(np.float32) ** 2 + in1.astype(np.float32) ** 2),
        ),
        subdim=False,
        uops_sha={},
    )
    dve_ops.OPS.append(op)
    idx = dve_ops._CUSTOM_DVE_ROW_BASE + len(dve_ops.OPS) - 1
    assert idx < 0x20
    dve_ops._SUB_OPCODE_FOR_NAME[op.name] = idx
    for ver in ("v3",):
        try:
            op.compile(ver)
        except ValueError as e:
            m = re.search(r"v3: ([0-9a-f]{16})", str(e))
            assert m, str(e)
            op.uops_sha[ver] = m.group(1)
            op.compile(ver)
    return op


SQ2 = _make_sq2()


def build():
    nc = bacc.Bacc("TRN2", target_bir_lowering=False, debug=False, num_devices=NCORES)

    def inp(name, shape):
        return nc.dram_tensor(name, shape, F32, kind="ExternalInput").ap()

    msa = inp("msa", [L, D])              # full msa (replicated)
    msa_my = inp("msa_my", [MYQ, D])      # this core's q rows
    pair_my = inp("pair_my", [MYQ, L, C])  # this core's pair slice
    ln_node_g = inp("ln_node_g", [1, D])
    ln_node_b = inp("ln_node_b", [1, D])
    ln_pair_g = inp("ln_pair_g", [C, 1])
    ln_pair_b = inp("ln_pair_b", [C, 1])
    row_Wq = inp("row_Wq", [D, D])
    row_Wk = inp("row_Wk", [D, D])
    row_Wv = inp("row_Wv", [D, D])
    row_Wb = inp("row_Wb", [C, H])
    row_Wg = inp("row_Wg", [D, D])
    row_bg = inp("row_bg", [1, D])
    row_Wo = inp("row_Wo", [D, D])
    row_bo = inp("row_bo", [1, D])
    col_Wq = inp("col_Wq", [D, D])
    col_Wk = inp("col_Wk", [D, D])
    col_Wv = inp("col_Wv", [D, D])
    col_Wg = inp("col_Wg", [D, D])
    col_bg = inp("col_bg", [1, D])
    col_Wo = inp("col_Wo", [D, D])
    col_bo = inp("col_bo", [1, D])
    ff_ln_g = inp("ff_ln_g", [1, D])
    ff_ln_b = inp("ff_ln_b", [1, D])
    ff_W1 = inp("ff_W1", [D, D])
    ff_b1 = inp("ff_b1", [D, 1])
    ff_W2 = inp("ff_W2", [D, D])
    ff_b2 = inp("ff_b2", [1, D])

    out_my = nc.dram_tensor("out_my", [MYQ, D], F32, kind="ExternalOutput").ap()

    gin = [nc.dram_tensor(f"gather_in{h}", [D, HALF], BF16) for h in range(2)]
    gout = [nc.dram_tensor(f"gather_out{h}", [NCORES * D, HALF], BF16,
                           addr_space="Shared") for h in range(2)]

    import os
    reps = int(os.environ.get("KREPS", "1"))
    with tile.TileContext(nc) as tc:
        for _ in range(reps):
            _body(nc, tc, locals())
    nc.compile()
    return nc


def _bcast(nc, pool, src_1xD, n_free=D, tag=None):
    t = pool.tile([128, n_free], F32, tag=tag, name=tag)
    src = bass.AP(tensor=src_1xD.tensor, offset=src_1xD.offset,
                  ap=[[0, 128], src_1xD.ap[-1]])
    nc.sync.dma_start(out=t, in_=src)
    return t


def _body(nc, tc, v):
    msa, msa_my, pair_my = v["msa"], v["msa_my"], v["pair_my"]
    out_my, gin, gout = v["out_my"], v["gin"], v["gout"]

    from contextlib import ExitStack
    ctx = ExitStack()
    pers = ctx.enter_context(tc.tile_pool(name="pers", bufs=1))
    roll = ctx.enter_context(tc.tile_pool(name="roll", bufs=2))
    nat_pool = ctx.enter_context(tc.tile_pool(name="nat", bufs=3))
    sq_pool = ctx.enter_context(tc.tile_pool(name="sq", bufs=2))
    pT_pool = ctx.enter_context(tc.tile_pool(name="pT", bufs=3))
    st_pool = ctx.enter_context(tc.tile_pool(name="st", bufs=2))
    roll3 = ctx.enter_context(tc.tile_pool(name="roll3", bufs=3))
    pp_tp = ctx.enter_context(tc.tile_pool(name="pp_tp", bufs=2, space="PSUM"))
    pp_s = ctx.enter_context(tc.tile_pool(name="pp_s", bufs=3, space="PSUM"))
    pp_l = ctx.enter_context(tc.tile_pool(name="pp_l", bufs=2, space="PSUM"))

    def P(shape, dt=F32, tag=None):
        return pers.tile(shape, dt, tag=tag, name=tag)

    # ---- constants needed by the first pair blocks ----
    ident = P([128, 128], BF16, tag="ident")
    make_identity(nc, ident)

    # ---- pair stream (gpsimd-only queue): prefetch 3 blocks now ----
    pair_g = pair_my.rearrange("q (p j) c -> p q j c", p=128)
    p_nats = []

    def issue_pair_dma(b):
        t = nat_pool.tile([128, QBLK, T, C], BF16, tag="p_nat", name="p_nat")
        nc.gpsimd.dma_start(out=t, in_=pair_g[:, b * QBLK:(b + 1) * QBLK, :, :])
        p_nats.append(t)

    for b in range(3):
        issue_pair_dma(b)

    ones_col = P([1, 128], F32, tag="ones_col")
    nc.vector.memset(ones_col, 1.0)
    ones128 = P([128, 1], F32, tag="ones128")
    nc.vector.memset(ones128, 1.0)
    ones_k = P([128, 1], BF16, tag="ones_k")
    nc.vector.memset(ones_k, 1.0)
    eps_t = P([128, 1], F32, tag="eps_t")
    nc.vector.memset(eps_t, (C ** 2) * EPS)
    eps_s = P([128, 1], F32, tag="eps_s")
    nc.vector.memset(eps_s, EPS)

    # ---- small sync loads for Wstat (critical path of block 0) ----
    Wb_sb = P([C, H], F32, tag="Wb_sb")
    nc.sync.dma_start(out=Wb_sb, in_=v["row_Wb"])
    g_pair = P([C, 1], F32, tag="g_pair")
    nc.sync.dma_start(out=g_pair, in_=v["ln_pair_g"])
    b_pair = P([C, 1], F32, tag="b_pair")
    nc.sync.dma_start(out=b_pair, in_=v["ln_pair_b"])

    Wgb = P([C, H], F32, tag="Wgb")
    nc.vector.tensor_scalar_mul(Wgb, Wb_sb, g_pair)
    ps_small = pp_l.tile([128, 128], F32, tag="ps_lg", name="ps_small")
    nc.tensor.matmul(ps_small[0:1, 0:H], ones128, Wgb, start=True, stop=True)
    u_row = P([1, H], F32, tag="u_row")
    nc.scalar.mul(u_row, ps_small[0:1, 0:H], -1.0)
    ps_small2 = pp_l.tile([128, 128], F32, tag="ps_lg", name="ps_small")
    nc.tensor.matmul(ps_small2[0:1, 0:H], b_pair, Wb_sb, start=True, stop=True)
    w_row = P([1, H], F32, tag="w_row")
    nc.scalar.copy(w_row, ps_small2[0:1, 0:H])
    ps_u = pp_l.tile([128, 128], F32, tag="ps_lg", name="ps_small")
    nc.tensor.matmul(ps_u[:, 0:H], ones_col, u_row, start=True, stop=True)
    Wgb_s = P([C, H], F32, tag="Wgb_s")
    nc.vector.tensor_scalar_mul(Wgb_s, Wgb, float(C))
    Wstat = P([C, H + 1], BF16, tag="Wstat")
    nc.vector.tensor_add(Wstat[:, 0:H], ps_u[:, 0:H], Wgb_s)
    nc.vector.tensor_copy(Wstat[:, H:H + 1], ones_k)
    ps_w = pp_l.tile([128, 128], F32, tag="ps_lg", name="ps_small")
    nc.tensor.matmul(ps_w[:, 0:H], ones_col, w_row, start=True, stop=True)
    w_tile = P([128, H], F32, tag="w_tile")
    nc.scalar.copy(w_tile, ps_w[:, 0:H])

    # ---- remaining sync loads (ordered by need) ----
    G_node = _bcast(nc, pers, v["ln_node_g"], tag="G_node")
    B_node = _bcast(nc, pers, v["ln_node_b"], tag="B_node")
    msa_g_t = roll.tile([128, T, D], F32, tag="msa_g", name="msa_g", bufs=1)
    nc.sync.dma_start(out=msa_g_t, in_=msa.rearrange("(p j) d -> p j d", p=128))
    # my q rows per half (partitions 0..63 each, so all half-ops stay aligned)
    msa_my_t = [roll.tile([128, D], F32, tag=f"msa_my{h}", name="msa_my", bufs=1)
                for h in range(2)]
    for h in range(2):
        nc.sync.dma_start(out=msa_my_t[h][0:HALF, :],
                          in_=msa_my[h * HALF:(h + 1) * HALF, :])

    weights = {}

    def wbf(name):
        stage = roll.tile([128, 2, D], F32, tag="wstage", name="wstage")
        nc.sync.dma_start(out=stage, in_=v[name].rearrange("(a p) d -> p a d", p=128))
        tl = P([128, 2, D], BF16, tag=f"w_{name}")
        nc.scalar.copy(tl, stage)
        weights[name] = tl
        return tl

    bias_rows = {}

    def load_bias(name):
        t = P([1, D], F32, tag=f"b_{name}")
        nc.sync.dma_start(out=t, in_=v[name])
        bias_rows[name] = t

    # ---- persistent state ----
    S_half = [P([128, T * HALF * H], BF16, tag=f"S_half{h}") for h in range(2)]
    x0_f = P([128, T, D], F32, tag="x0_f")
    x0_bf = P([128, T, D], BF16, tag="x0_bf")
    x0my_f = [P([128, D], F32, tag=f"x0my_f{h}") for h in range(2)]
    x0my_bf = [P([128, D], BF16, tag=f"x0my_bf{h}") for h in range(2)]
    x0T = P([128, 2, L], BF16, tag="x0T")
    x0Tmy = P([128, 2, 128], BF16, tag="x0Tmy")
    KT_row = P([128, 2, L], BF16, tag="KT_row")
    QT_row = P([128, 2, 128], BF16, tag="QT_row")
    V_row = P([128, T, D], BF16, tag="V_row")
    x1_f = [P([128, D], F32, tag=f"x1_f{h}") for h in range(2)]
    x1_bf = [P([128, D], BF16, tag=f"x1_bf{h}") for h in range(2)]
    o_row = [P([128, D], BF16, tag=f"o_row{h}") for h in range(2)]
    x1Tmy = P([128, 2, 128], BF16, tag="x1Tmy")
    x1T = [P([128, 2, 512], BF16, tag=f"x1T{h}") for h in range(2)]
    KT_col = P([128, 2, L], BF16, tag="KT_col")
    V_col = P([128, T, D], BF16, tag="V_col")

    def layer_norm(dst_f32, dst_bf, xt, g_t, b_t, pool, n=128):
        st = pool.tile([128, 6], F32, tag="ln_st", name="ln_st")
        nc.vector.bn_stats(st[0:n, :], xt)
        mv = pool.tile([128, 2], F32, tag="ln_mv", name="ln_mv")
        nc.vector.bn_aggr(mv[0:n, :], st[0:n, :])
        sq = pool.tile([128, 1], F32, tag="ln_sq", name="ln_sq")
        nc.scalar.activation(sq[0:n, :], mv[0:n, 1:2], AF.Sqrt, bias=eps_s[0:n, :], scale=1.0)
        r = pool.tile([128, 1], F32, tag="ln_r", name="ln_r")
        nc.vector.reciprocal(r[0:n, :], sq[0:n, :])
        mr = pool.tile([128, 1], F32, tag="ln_mr", name="ln_mr")
        nc.vector.tensor_mul(mr[0:n, :], mv[0:n, 0:1], r[0:n, :])
        nmr = pool.tile([128, 1], F32, tag="ln_nmr", name="ln_nmr")
        nc.vector.tensor_scalar_mul(nmr[0:n, :], mr[0:n, :], -1.0)
        xn = pool.tile([128, D], F32, tag="ln_xn", name="ln_xn")
        nc.scalar.activation(xn[0:n, :], xt, AF.Identity, bias=nmr[0:n, :], scale=r[0:n, :])
        nc.vector.tensor_mul(dst_f32, xn[0:n, :], g_t[0:n, :])
        nc.vector.tensor_add(dst_f32, dst_f32, b_t[0:n, :])
        nc.vector.tensor_copy(dst_bf, dst_f32)

    def transpose_to(dst_bf, src_tiles, n, ncol=128):
        ps = pp_tp.tile([128, T * 128], BF16, tag="tp", name="tp")
        for i in range(n):
            nc.tensor.transpose(ps[:, i * ncol:(i + 1) * ncol], src_tiles[i],
                                ident[0:ncol, 0:ncol])
        nc.scalar.copy(dst_bf[:, 0:n * ncol], ps[:, 0:n * ncol])

    def project_T(dst, W_bf, xT_full, n_l, scale=None, c0=0, src_c0=None):
        if src_c0 is None:
            src_c0 = c0
        for jm in range(2):
            for q4 in range(0, n_l, 256):
                w = min(256, n_l - q4)
                ps = pp_s.tile([128, 288], F32, tag="proj", name="proj")
                for Dj in range(2):
                    nc.tensor.matmul(
                        ps[:, 0:w],
                        W_bf[:, Dj, jm * 128:(jm + 1) * 128],
                        xT_full[:, Dj, src_c0 + q4:src_c0 + q4 + w],
                        start=(Dj == 0), stop=(Dj == 1))
                if scale is None:
                    nc.scalar.copy(dst[:, jm, c0 + q4:c0 + q4 + w], ps[:, 0:w])
                else:
                    nc.scalar.mul(dst[:, jm, c0 + q4:c0 + q4 + w], ps[:, 0:w], scale)

    def project_V(dst, W_bf, xT_full, tiles, src_base=0):
        for ti, t in enumerate(tiles):
            for dh in range(0, D, 256):
                ps = pp_s.tile([128, 288], F32, tag="proj", name="proj")
                for Dj in range(2):
                    nc.tensor.matmul(
                        ps[:, 0:256],
                        xT_full[:, Dj, src_base + ti * 128:src_base + (ti + 1) * 128],
                        W_bf[:, Dj, dh:dh + 256],
                        start=(Dj == 0), stop=(Dj == 1))
                nc.scalar.copy(dst[:, t, dh:dh + 256], ps[:, 0:256])

    # ================= pair block =================
    def pair_block(b):
        p_nat = p_nats[b]
        flat = p_nat.rearrange("p q j c -> p (q j) c")
        t_sq = sq_pool.tile([128, QBLK * T, C // 2], BF16, tag="t_sq", name="t_sq")
        nc.vector._custom_dve(SQ2, out=t_sq, in0=flat[:, :, 0:C // 2],
                              in1=flat[:, :, C // 2:C])
        u_sq = sq_pool.tile([128, QBLK * T, C // 4], BF16, tag="u_sq", name="u_sq")
        nc.vector.tensor_add(u_sq, t_sq[:, :, 0:C // 4], t_sq[:, :, C // 4:C // 2])
        sumsq = st_pool.tile([128, QBLK * T], F32, tag="sumsq", name="sumsq")
        nc.vector.tensor_reduce(out=sumsq, in_=u_sq, axis=AX.X, op=OP.add)

        sums = st_pool.tile([128, QBLK * T], F32, tag="sums", name="sums")
        ps_S_list = []
        for hb in range(2):
            ps_S = pp_s.tile([128, 288], F32, tag="proj", name="ps_S")
            ps_S_list.append(ps_S)
            for qi in range(4):
                qq = hb * 4 + qi
                ps_t = pp_tp.tile([128, T * 128], BF16, tag="tp", name="tp")
                for j in range(T):
                    nc.tensor.transpose(ps_t[:, j * 128:(j + 1) * 128],
                                        p_nat[:, qq, j, :], ident)
                pT = pT_pool.tile([128, T * 128], BF16, tag="pT", name="pT")
                nc.scalar.copy(pT, ps_t)
                for j in range(T):
                    nc.tensor.matmul(
                        ps_S[:, (qi * T + j) * 9:(qi * T + j) * 9 + 9],
                        pT[:, j * 128:(j + 1) * 128], Wstat,
                        start=True, stop=True)
            nc.vector.tensor_copy(
                sums[:, hb * 32:(hb + 1) * 32],
                bass.AP(tensor=ps_S.tensor, offset=ps_S.offset + 8,
                        ap=[ps_S.ap[0], [9, 32]]))

        t1 = st_pool.tile([128, QBLK * T], F32, tag="t1", name="t1")
        nc.vector.tensor_mul(t1, sums, sums)
        v128 = st_pool.tile([128, QBLK * T], F32, tag="v128", name="v128")
        nc.vector.tensor_scalar_mul(v128, sumsq, float(C))
        nc.vector.tensor_sub(v128, v128, t1)
        sqv = st_pool.tile([128, QBLK * T], F32, tag="sqv", name="sqv")
        nc.scalar.activation(sqv, v128, AF.Sqrt, bias=eps_t, scale=1.0)
        r_all = st_pool.tile([128, QBLK * T], F32, tag="r_all", name="r_all")
        nc.vector.reciprocal(r_all, sqv)

        # assembly into the right half: S_h[p, j*HALF*H + q'*H + h] * r
        S_dst = S_half[b // (NBLK // 2)]
        qbase = (b % (NBLK // 2)) * QBLK
        for hb in range(2):
            ps_S = ps_S_list[hb]
            out_ap = bass.AP(
                tensor=S_dst.tensor,
                offset=S_dst.offset + (qbase + hb * 4) * H,
                ap=[S_dst.ap[0], [H, 4], [HALF * H, T], [1, H]])
            in_ap = bass.AP(
                tensor=ps_S.tensor, offset=ps_S.offset,
                ap=[ps_S.ap[0], [9 * T, 4], [9, T], [1, H]])
            r_ap = bass.AP(
                tensor=r_all.tensor, offset=r_all.offset + hb * 4 * T,
                ap=[r_all.ap[0], [T, 4], [1, T], [0, H]])
            nc.vector.tensor_tensor(out=out_ap, in0=in_ap, in1=r_ap, op=OP.mult)

    # ================= attention / gate (per 64-q half) =================
    def attn_head(h, h8, KT, QT, V, S_bias, w_t, o_half):
        """One head of k-part attention for half h; o_half [64, D] bf16."""
        h0 = h * HALF
        ps_o = pp_l.tile([128, 128], F32, tag="ps_o", name="ps_o", bufs=1)
        E = roll3.tile([128, T * HALF], BF16, tag="E", name="E")
        for j in range(T):
            ps_lg = pp_l.tile([128, 128], F32, tag="ps_lg", name="ps_lg")
            jh, rh = h8 // 4, (h8 % 4) * 32
            nc.tensor.matmul(
                ps_lg[:, 0:HALF],
                KT[rh:rh + 32, jh, j * 128:(j + 1) * 128],
                QT[rh:rh + 32, jh, h0:h0 + HALF],
                start=True, stop=(S_bias is None),
                tile_position=(rh, 0))
            if S_bias is not None:
                bias_ap = bass.AP(
                    tensor=S_bias.tensor,
                    offset=S_bias.offset + j * HALF * H + h8,
                    ap=[S_bias.ap[0], [H, HALF]])
                nc.tensor.matmul(ps_lg[:, 0:HALF], ident, bias_ap,
                                 start=False, stop=True)
            if w_t is not None:
                nc.scalar.activation(E[:, j * HALF:(j + 1) * HALF], ps_lg[:, 0:HALF],
                                     AF.Exp, bias=w_t[:, h8:h8 + 1], scale=1.0)
            else:
                nc.scalar.activation(E[:, j * HALF:(j + 1) * HALF], ps_lg[:, 0:HALF],
                                     AF.Exp, bias=0.0, scale=1.0)
        for j in range(T):
            nc.tensor.matmul(ps_o[0:HALF, 0:DH], E[:, j * HALF:(j + 1) * HALF],
                             V[:, j, h8 * DH:(h8 + 1) * DH],
                             start=(j == 0), stop=False)
            nc.tensor.matmul(ps_o[0:HALF, DH:DH + 1], E[:, j * HALF:(j + 1) * HALF],
                             ones_k, start=(j == 0), stop=(j == T - 1))
        recip = roll3.tile([128, 1], F32, tag="recip", name="recip")
        nc.vector.reciprocal(recip[0:HALF, :], ps_o[0:HALF, DH:DH + 1])
        nc.vector.tensor_scalar_mul(o_half[0:HALF, h8 * DH:(h8 + 1) * DH],
                                    ps_o[0:HALF, 0:DH], recip[0:HALF, :])

    def gate_proj_residual_half(h0, xT_my, Wg_bf, bg_row, Wo_bf, bo_row, o_half,
                                x_prev_half, x_new_f, x_new_bf, hpool):
        ps_g = pp_s.tile([128, 288], F32, tag="proj", name="proj")
        for Dj in range(2):
            nc.tensor.matmul(ps_g[0:HALF, 0:256], xT_my[:, Dj, h0:h0 + HALF],
                             Wg_bf[:, Dj, :], start=(Dj == 0), stop=False)
        nc.tensor.matmul(ps_g[0:HALF, 0:256], ones_col[:, 0:HALF], bg_row,
                         start=False, stop=True)
        g_sb = hpool.tile([128, D], BF16, tag="g_sb", name="g_sb")
        nc.scalar.activation(g_sb[0:HALF, :], ps_g[0:HALF, 0:256], AF.Sigmoid,
                             bias=0.0, scale=1.0)
        go = hpool.tile([128, D], BF16, tag="go", name="go")
        nc.vector.tensor_mul(go[0:HALF, :], g_sb[0:HALF, :], o_half[0:HALF, :])
        goT = hpool.tile([128, 2, HALF], BF16, tag="goT", name="goT")
        for jm in range(2):
            transpose_to(goT[:, jm, :], [go[0:HALF, jm * 128:(jm + 1) * 128]], 1,
                         ncol=HALF)
        ps_y = pp_s.tile([128, 288], F32, tag="proj", name="proj")
        for Dj in range(2):
            nc.tensor.matmul(ps_y[0:HALF, 0:256], goT[:, Dj, 0:HALF],
                             Wo_bf[:, Dj, :], start=(Dj == 0), stop=False)
        nc.tensor.matmul(ps_y[0:HALF, 0:256], ones_col[:, 0:HALF], bo_row,
                         start=False, stop=True)
        nc.vector.tensor_add(x_new_f[0:HALF, :], x_prev_half[0:HALF, :],
                             ps_y[0:HALF, 0:256])
        nc.vector.tensor_copy(x_new_bf[0:HALF, :], x_new_f[0:HALF, :])

    def row_gate_and_gather(h):
        h0 = h * HALF
        gate_proj_residual_half(h0, x0Tmy, weights["row_Wg"], bias_rows["row_bg"],
                                weights["row_Wo"], bias_rows["row_bo"], o_row[h],
                                x0my_f[h], x1_f[h], x1_bf[h], roll3)
        for jm in range(2):
            transpose_to(x1Tmy[:, jm, h0:h0 + HALF],
                         [x1_bf[h][0:HALF, jm * 128:(jm + 1) * 128]], 1, ncol=HALF)
        for jm in range(2):
            nc.sync.dma_start(out=gin[h].ap()[jm * 128:(jm + 1) * 128, :],
                              in_=x1Tmy[:, jm, h0:h0 + HALF])
        nc.gpsimd.collective_compute(
            "AllGather", OP.bypass,
            replica_groups=[list(range(NCORES))],
            ins=[gin[h].ap().opt()],
            outs=[gout[h].ap().opt()])

    def col_kv_for_half(h):
        # read back AG#h -> x1T[h] cols (i, q'), then project K/V tiles
        gout_r = gout[h].ap().rearrange("(i a p) q -> p a i q", i=NCORES, a=2)
        x1T_4d = x1T[h].rearrange("p a (i q) -> p a i q", i=NCORES)
        for jm in range(2):
            nc.sync.dma_start(out=x1T_4d[:, jm, :, :], in_=gout_r[:, jm, :, :])
        project_T(KT_col, weights["col_Wk"], x1T[h], 512, c0=h * 512, src_c0=0)
        project_V(V_col, weights["col_Wv"], x1T[h], range(4 * h, 4 * h + 4))

    # ================= interleaved schedule =================
    def setup_step(b):
        if b == 0:
            for j in range(T):
                layer_norm(x0_f[:, j, :], x0_bf[:, j, :], msa_g_t[:, j, :],
                           G_node, B_node, roll)
            for h in range(2):
                layer_norm(x0my_f[h][0:HALF, :], x0my_bf[h][0:HALF, :],
                           msa_my_t[h][0:HALF, :], G_node, B_node, roll, n=HALF)
        elif b == 1:
            wbf("row_Wk")
            wbf("row_Wq")
            for jm in range(2):
                transpose_to(x0T[:, jm, :],
                             [x0_bf[:, j, jm * 128:(jm + 1) * 128] for j in range(T)], T)
            for h in range(2):
                for jm in range(2):
                    transpose_to(x0Tmy[:, jm, h * HALF:(h + 1) * HALF],
                                 [x0my_bf[h][0:HALF, jm * 128:(jm + 1) * 128]], 1,
                                 ncol=HALF)
        elif b == 2:
            project_T(KT_row, weights["row_Wk"], x0T, L)
            project_T(QT_row, weights["row_Wq"], x0Tmy, 128, scale=SCALE)
        elif b == 3:
            wbf("row_Wv")
            project_V(V_row, weights["row_Wv"], x0T, range(T))
        elif b == 4:
            wbf("row_Wg")
            wbf("row_Wo")
            load_bias("row_bg")
            load_bias("row_bo")
        elif b == 5:
            wbf("col_Wk")
            wbf("col_Wv")
        elif b == 6:
            wbf("col_Wq")
            wbf("col_Wg")
            wbf("col_Wo")
            load_bias("col_bg")
            load_bias("col_bo")
        elif b == 7:
            wbf("ff_W1")
            wbf("ff_W2")
            load_bias("ff_b2")
            b1T = P([128, 2], F32, tag="b1T")
            nc.sync.dma_start(out=b1T, in_=v["ff_b1"].rearrange("(a p) o -> p (a o)", p=128))
            bias_rows["ff_b1T"] = b1T
            g = _bcast(nc, pers, v["ff_ln_g"], tag="G_ff")
            bias_rows["G_ff"] = g
            bb = _bcast(nc, pers, v["ff_ln_b"], tag="B_ff")
            bias_rows["B_ff"] = bb
        elif 8 <= b <= 11:
            for h8 in range(2 * (b - 8), 2 * (b - 8) + 2):
                attn_head(0, h8, KT_row, QT_row, V_row, S_half[0], w_tile, o_row[0])
        elif b == 12:
            row_gate_and_gather(0)
        elif b == 14:
            col_kv_for_half(0)

    for b in range(NBLK):
        if b + 3 < NBLK:
            issue_pair_dma(b + 3)
        pair_block(b)
        setup_step(b)

    # ---- row attention half 1 + AG#2 ----
    for h8 in range(H):
        attn_head(1, h8, KT_row, QT_row, V_row, S_half[1], w_tile, o_row[1])
    row_gate_and_gather(1)
    col_kv_for_half(1)

    # ---- col attention + FF per half ----
    QT_col = P([128, 2, 128], BF16, tag="QT_col")
    project_T(QT_col, weights["col_Wq"], x1Tmy, 128, scale=SCALE)

    for h in range(2):
        h0 = h * HALF
        o_col = roll3.tile([128, D], BF16, tag="o_col", name="o_col")
        for h8 in range(H):
            attn_head(h, h8, KT_col, QT_col, V_col, None, None, o_col)
        x2_f = roll3.tile([128, D], F32, tag="x2_f", name="x2_f")
        x2_bf = roll3.tile([128, D], BF16, tag="x2_bf", name="x2_bf")
        gate_proj_residual_half(h0, x1Tmy, weights["col_Wg"], bias_rows["col_bg"],
                                weights["col_Wo"], bias_rows["col_bo"], o_col,
                                x1_f[h], x2_f, x2_bf, roll3)

        h_f = roll3.tile([128, D], F32, tag="h_f", name="h_f")
        h_bf = roll3.tile([128, D], BF16, tag="h_bf", name="h_bf")
        layer_norm(h_f[0:HALF, :], h_bf[0:HALF, :], x2_f[0:HALF, :],
                   bias_rows["G_ff"], bias_rows["B_ff"], roll3, n=HALF)
        hT = roll3.tile([128, 2, HALF], BF16, tag="hT", name="hT")
        for jm in range(2):
            transpose_to(hT[:, jm, :], [h_bf[0:HALF, jm * 128:(jm + 1) * 128]], 1,
                         ncol=HALF)
        a1T = roll3.tile([128, 2, HALF], BF16, tag="a1T", name="a1T")
        for jm in range(2):
            ps_z = pp_s.tile([128, 288], F32, tag="proj", name="proj")
            for Dj in range(2):
                nc.tensor.matmul(ps_z[:, 0:HALF],
                                 weights["ff_W1"][:, Dj, jm * 128:(jm + 1) * 128],
                                 hT[:, Dj, :], start=(Dj == 0), stop=(Dj == 1))
            nc.scalar.activation(a1T[:, jm, :], ps_z[:, 0:HALF], AF.Relu,
                                 bias=bias_rows["ff_b1T"][:, jm:jm + 1], scale=1.0)
        ps_y = pp_s.tile([128, 288], F32, tag="proj", name="proj")
        for jm in range(2):
            nc.tensor.matmul(ps_y[0:HALF, 0:256], a1T[:, jm, :],
                             weights["ff_W2"][:, jm, :],
                             start=(jm == 0), stop=False)
        nc.tensor.matmul(ps_y[0:HALF, 0:256], ones_col[:, 0:HALF],
                         bias_rows["ff_b2"], start=False, stop=True)
        out_sb = roll3.tile([128, D], F32, tag="out_sb", name="out_sb")
        nc.vector.tensor_add(out_sb[0:HALF, :], x2_f[0:HALF, :], ps_y[0:HALF, 0:256])
        nc.sync.dma_start(out=out_my[h0:h0 + HALF, :], in_=out_sb[0:HALF, :])
    ctx.close()


_NC_CACHE = None


def make_in_maps(common, msa, pair):
    in_maps = []
    for i in range(NCORES):
        m = dict(common)
        m["msa_my"] = np.ascontiguousarray(msa[i * MYQ:(i + 1) * MYQ, :])
        m["pair_my"] = np.ascontiguousarray(pair[i * MYQ:(i + 1) * MYQ, :, :])
        in_maps.append(m)
    return in_maps


def kernel(**inputs):
    global _NC_CACHE
    if _NC_CACHE is None:
        _NC_CACHE = build()
    nc = _NC_CACHE

    msa = np.asarray(inputs["msa"]).reshape(L, D).astype(np.float32)
    pair = np.asarray(inputs["pair"]).reshape(L, L, C).astype(np.float32)

    def f(name, shape):
        return np.ascontiguousarray(
            np.asarray(inputs[name]).reshape(shape).astype(np.float32))

    common = {
        "msa": msa,
        "ln_node_g": f("ln_node_g", (1, D)), "ln_node_b": f("ln_node_b", (1, D)),
        "ln_pair_g": f("ln_pair_g", (C, 1)), "ln_pair_b": f("ln_pair_b", (C, 1)),
        "row_Wq": f("row_Wq", (D, D)), "row_Wk": f("row_Wk", (D, D)),
        "row_Wv": f("row_Wv", (D, D)), "row_Wb": f("row_Wb", (C, H)),
        "row_Wg": f("row_Wg", (D, D)), "row_bg": f("row_bg", (1, D)),
        "row_Wo": f("row_Wo", (D, D)), "row_bo": f("row_bo", (1, D)),
        "col_Wq": f("col_Wq", (D, D)), "col_Wk": f("col_Wk", (D, D)),
        "col_Wv": f("col_Wv", (D, D)),
        "col_Wg": f("col_Wg", (D, D)), "col_bg": f("col_bg", (1, D)),
        "col_Wo": f("col_Wo", (D, D)), "col_bo": f("col_bo", (1, D)),
        "ff_ln_g": f("ff_ln_g", (1, D)), "ff_ln_b": f("ff_ln_b", (1, D)),
        "ff_W1": f("ff_W1", (D, D)), "ff_b1": f("ff_b1", (D, 1)),
        "ff_W2": f("ff_W2", (D, D)), "ff_b2": f("ff_b2", (1, D)),
    }
    in_maps = make_in_maps(common, msa, pair)
    res = run_bass_kernel_spmd(nc, in_maps, core_ids=list(range(NCORES)))
    out = np.concatenate([res.results[i]["out_my"] for i in range(NCORES)], axis=0)
    return out.reshape(1, L, D).astype(np.float32)


if __name__ == "__main__":
    build()
    print("build OK")


# revision 12
# speedup vs baseline: 1.6487x; 1.1622x over previous
"""AlphaFold-style node update (row-gated-attn + col-gated-attn + FF) on 8 TRN2 cores.

Sharding: L (query rows) across 8 cores, weights replicated.  v3 pipeline:
  - pair streamed f32->bf16 via SWDGE cast-DMA in a grouped-k layout
    (partition p holds keys k=8p+j -> 4KB contiguous lines); the permutation
    is applied consistently to K/V via the same grouped msa load, and
    attention is permutation-invariant over keys.
  - per block (8 q x 1024 k): fused custom DVE op t=a^2+b^2 (halves the
    square+reduce stream), add-fold, batched reduce -> sumsq; PE transposes
    [k,c]->[c,k]; ACT PSUM->SBUF copies; per-j S-matmuls vs Wstat (pair-bias
    projection + row-sum column); r = rsqrt chain; DVE assembly of
    S_half[k, (j q h)] * r.
  - setup (LN/x0T/K/V/Q projections, weight casts) interleaved into the
    first pair blocks; weights loaded f32 on HWDGE + ACT cast so gpsimd
    serves only the pair stream.
  - row attention / gate / residual per 64-q half: heads spread across pair
    blocks 8..11, AllGather of x1^T per half (first AG hidden under the
    remaining pair blocks); col attention + FF per half at the tail.
"""
import re
import numpy as np

import concourse.bass as bass
import concourse.bacc as bacc
import concourse.tile as tile
from concourse import mybir
from concourse import dve_ops
from concourse.dve_ops import DveOp
from concourse.dve_spec import Spec, Src0, Src1
from concourse.bass_utils import run_bass_kernel_spmd
from concourse.masks import make_identity

F32 = mybir.dt.float32
BF16 = mybir.dt.bfloat16
AX = mybir.AxisListType
OP = mybir.AluOpType
AF = mybir.ActivationFunctionType

NCORES = 8
L = 1024          # sequence length
D = 256           # d_msa
C = 128           # d_pair
H = 8             # heads
DH = 32           # head dim
MYQ = L // NCORES  # 128 q rows per core
T = L // 128      # 8 k-tiles (tile j holds keys k = 8p + j)
SCALE = 1.0 / float(np.sqrt(DH))
EPS = 1e-5
QBLK = 8          # q rows per pair-loop block
NBLK = MYQ // QBLK  # 16 blocks
HALF = MYQ // 2   # 64 q rows per attention half


def _make_sq2():
    """Custom DVE op: out = Src0^2 + Src1^2 (fused square + pairwise fold)."""
    for op in dve_ops.OPS:
        if op.name == "SQ2_ADD_ANT":
            return op
    op = DveOp(
        "SQ2_ADD_ANT",
        Spec(
            body=Src0 * Src0 + Src1 * Src1,
            reference=lambda in0, in1, s0, s1, imm2: (
                in0.astype(np.float32) ** 2 + in1.astype(np.float32) ** 2),
        ),
        subdim=False,
        uops_sha={},
    )
    dve_ops.OPS.append(op)
    idx = dve_ops._CUSTOM_DVE_ROW_BASE + len(dve_ops.OPS) - 1
    assert idx < 0x20
    dve_ops._SUB_OPCODE_FOR_NAME[op.name] = idx
    for ver in ("v3",):
        try:
            op.compile(ver)
        except ValueError as e:
            m = re.search(r"v3: ([0-9a-f]{16})", str(e))
            assert m, str(e)
            op.uops_sha[ver] = m.group(1)
            op.compile(ver)
    return op


SQ2 = _make_sq2()


def build():
    nc = bacc.Bacc("TRN2", target_bir_lowering=False, debug=False, num_devices=NCORES)

    def inp(name, shape):
        return nc.dram_tensor(name, shape, F32, kind="ExternalInput").ap()

    msa = inp("msa", [L, D])              # full msa (replicated)
    msa_my = inp("msa_my", [MYQ, D])      # this core's q rows
    pair_my = inp("pair_my", [MYQ, L, C])  # this core's pair slice
    ln_node_g = inp("ln_node_g", [1, D])
    ln_node_b = inp("ln_node_b", [1, D])
    ln_pair_g = inp("ln_pair_g", [C, 1])
    ln_pair_b = inp("ln_pair_b", [C, 1])
    row_Wq = inp("row_Wq", [D, D])
    row_Wk = inp("row_Wk", [D, D])
    row_Wv = inp("row_Wv", [D, D])
    row_Wb = inp("row_Wb", [C, H])
    row_Wg = inp("row_Wg", [D, D])
    row_bg = inp("row_bg", [1, D])
    row_Wo = inp("row_Wo", [D, D])
    row_bo = inp("row_bo", [1, D])
    col_Wq = inp("col_Wq", [D, D])
    col_Wk = inp("col_Wk", [D, D])
    col_Wv = inp("col_Wv", [D, D])
    col_Wg = inp("col_Wg", [D, D])
    col_bg = inp("col_bg", [1, D])
    col_Wo = inp("col_Wo", [D, D])
    col_bo = inp("col_bo", [1, D])
    ff_ln_g = inp("ff_ln_g", [1, D])
    ff_ln_b = inp("ff_ln_b", [1, D])
    ff_W1 = inp("ff_W1", [D, D])
    ff_b1 = inp("ff_b1", [D, 1])
    ff_W2 = inp("ff_W2", [D, D])
    ff_b2 = inp("ff_b2", [1, D])

    out_my = nc.dram_tensor("out_my", [MYQ, D], F32, kind="ExternalOutput").ap()

    gin = [nc.dram_tensor(f"gather_in{h}", [D, HALF], BF16) for h in range(2)]
    gout = [nc.dram_tensor(f"gather_out{h}", [NCORES * D, HALF], BF16,
                           addr_space="Shared") for h in range(2)]

    import os
    reps = int(os.environ.get("KREPS", "1"))
    with tile.TileContext(nc) as tc:
        for _ in range(reps):
            _body(nc, tc, locals())
    nc.compile()
    return nc


def _bcast(nc, pool, src_1xD, n_free=D, tag=None):
    t = pool.tile([128, n_free], F32, tag=tag, name=tag)
    src = bass.AP(tensor=src_1xD.tensor, offset=src_1xD.offset,
                  ap=[[0, 128], src_1xD.ap[-1]])
    nc.sync.dma_start(out=t, in_=src)
    return t


def _body(nc, tc, v):
    msa, msa_my, pair_my = v["msa"], v["msa_my"], v["pair_my"]
    out_my, gin, gout = v["out_my"], v["gin"], v["gout"]

    from contextlib import ExitStack
    ctx = ExitStack()
    pers = ctx.enter_context(tc.tile_pool(name="pers", bufs=1))
    roll = ctx.enter_context(tc.tile_pool(name="roll", bufs=2))
    nat_pool = ctx.enter_context(tc.tile_pool(name="nat", bufs=3))
    sq_pool = ctx.enter_context(tc.tile_pool(name="sq", bufs=2))
    pT_pool = ctx.enter_context(tc.tile_pool(name="pT", bufs=3))
    st_pool = ctx.enter_context(tc.tile_pool(name="st", bufs=2))
    roll3 = ctx.enter_context(tc.tile_pool(name="roll3", bufs=3))
    pp_tp = ctx.enter_context(tc.tile_pool(name="pp_tp", bufs=2, space="PSUM"))
    pp_s = ctx.enter_context(tc.tile_pool(name="pp_s", bufs=3, space="PSUM"))
    pp_l = ctx.enter_context(tc.tile_pool(name="pp_l", bufs=2, space="PSUM"))

    def P(shape, dt=F32, tag=None):
        return pers.tile(shape, dt, tag=tag, name=tag)

    # ---- constants needed by the first pair blocks ----
    ident = P([128, 128], BF16, tag="ident")
    make_identity(nc, ident)

    # ---- pair stream (gpsimd-only queue): prefetch 3 blocks now ----
    pair_g = pair_my.rearrange("q (p j) c -> p q j c", p=128)
    p_nats = []

    def issue_pair_dma(b):
        t = nat_pool.tile([128, QBLK, T, C], BF16, tag="p_nat", name="p_nat")
        nc.gpsimd.dma_start(out=t, in_=pair_g[:, b * QBLK:(b + 1) * QBLK, :, :])
        p_nats.append(t)

    for b in range(3):
        issue_pair_dma(b)

    ones_col = P([1, 128], F32, tag="ones_col")
    nc.vector.memset(ones_col, 1.0)
    ones128 = P([128, 1], F32, tag="ones128")
    nc.vector.memset(ones128, 1.0)
    ones_k = P([128, 1], BF16, tag="ones_k")
    nc.vector.memset(ones_k, 1.0)
    eps_t = P([128, 1], F32, tag="eps_t")
    nc.vector.memset(eps_t, (C ** 2) * EPS)
    eps_s = P([128, 1], F32, tag="eps_s")
    nc.vector.memset(eps_s, EPS)

    # ---- small sync loads for Wstat (critical path of block 0) ----
    Wb_sb = P([C, H], F32, tag="Wb_sb")
    nc.sync.dma_start(out=Wb_sb, in_=v["row_Wb"])
    g_pair = P([C, 1], F32, tag="g_pair")
    nc.sync.dma_start(out=g_pair, in_=v["ln_pair_g"])
    b_pair = P([C, 1], F32, tag="b_pair")
    nc.sync.dma_start(out=b_pair, in_=v["ln_pair_b"])

    Wgb = P([C, H], F32, tag="Wgb")
    nc.vector.tensor_scalar_mul(Wgb, Wb_sb, g_pair)
    ps_small = pp_l.tile([128, 128], F32, tag="ps_lg", name="ps_small")
    nc.tensor.matmul(ps_small[0:1, 0:H], ones128, Wgb, start=True, stop=True)
    u_row = P([1, H], F32, tag="u_row")
    nc.scalar.mul(u_row, ps_small[0:1, 0:H], -1.0)
    ps_small2 = pp_l.tile([128, 128], F32, tag="ps_lg", name="ps_small")
    nc.tensor.matmul(ps_small2[0:1, 0:H], b_pair, Wb_sb, start=True, stop=True)
    w_row = P([1, H], F32, tag="w_row")
    nc.scalar.copy(w_row, ps_small2[0:1, 0:H])
    ps_u = pp_l.tile([128, 128], F32, tag="ps_lg", name="ps_small")
    nc.tensor.matmul(ps_u[:, 0:H], ones_col, u_row, start=True, stop=True)
    Wgb_s = P([C, H], F32, tag="Wgb_s")
    nc.vector.tensor_scalar_mul(Wgb_s, Wgb, float(C))
    Wstat = P([C, H + 1], BF16, tag="Wstat")
    nc.vector.tensor_add(Wstat[:, 0:H], ps_u[:, 0:H], Wgb_s)
    nc.vector.tensor_copy(Wstat[:, H:H + 1], ones_k)
    ps_w = pp_l.tile([128, 128], F32, tag="ps_lg", name="ps_small")
    nc.tensor.matmul(ps_w[:, 0:H], ones_col, w_row, start=True, stop=True)
    w_tile = P([128, H], F32, tag="w_tile")
    nc.scalar.copy(w_tile, ps_w[:, 0:H])

    # ---- remaining sync loads (ordered by need) ----
    G_node = _bcast(nc, pers, v["ln_node_g"], tag="G_node")
    B_node = _bcast(nc, pers, v["ln_node_b"], tag="B_node")
    msa_g_t = roll.tile([128, T, D], F32, tag="msa_g", name="msa_g", bufs=1)
    nc.sync.dma_start(out=msa_g_t, in_=msa.rearrange("(p j) d -> p j d", p=128))
    # my q rows per half (partitions 0..63 each, so all half-ops stay aligned)
    msa_my_t = [roll.tile([128, D], F32, tag=f"msa_my{h}", name="msa_my", bufs=1)
                for h in range(2)]
    for h in range(2):
        nc.sync.dma_start(out=msa_my_t[h][0:HALF, :],
                          in_=msa_my[h * HALF:(h + 1) * HALF, :])

    weights = {}

    def wbf(name):
        stage = roll.tile([128, 2, D], F32, tag="wstage", name="wstage")
        nc.sync.dma_start(out=stage, in_=v[name].rearrange("(a p) d -> p a d", p=128))
        tl = P([128, 2, D], BF16, tag=f"w_{name}")
        nc.scalar.copy(tl, stage)
        weights[name] = tl
        return tl

    bias_rows = {}

    def load_bias(name):
        t = P([1, D], F32, tag=f"b_{name}")
        nc.sync.dma_start(out=t, in_=v[name])
        bias_rows[name] = t

    # ---- persistent state ----
    S_half = [P([128, T * HALF * H], BF16, tag=f"S_half{h}") for h in range(2)]
    x0_f = P([128, T, D], F32, tag="x0_f")
    x0_bf = P([128, T, D], BF16, tag="x0_bf")
    x0my_f = [P([128, D], F32, tag=f"x0my_f{h}") for h in range(2)]
    x0my_bf = [P([128, D], BF16, tag=f"x0my_bf{h}") for h in range(2)]
    x0T = P([128, 2, L], BF16, tag="x0T")
    x0Tmy = P([128, 2, 128], BF16, tag="x0Tmy")
    KT_row = P([128, 2, L], BF16, tag="KT_row")
    QT_row = P([128, 2, 128], BF16, tag="QT_row")
    V_row = P([128, T, D], BF16, tag="V_row")
    x1_f = [P([128, D], F32, tag=f"x1_f{h}") for h in range(2)]
    x1_bf = [P([128, D], BF16, tag=f"x1_bf{h}") for h in range(2)]
    o_row = [P([128, D], BF16, tag=f"o_row{h}") for h in range(2)]
    x1Tmy = P([128, 2, 128], BF16, tag="x1Tmy")
    x1T = [P([128, 2, 512], BF16, tag=f"x1T{h}") for h in range(2)]
    KT_col = P([128, 2, L], BF16, tag="KT_col")
    V_col = P([128, T, D], BF16, tag="V_col")

    def layer_norm(dst_f32, dst_bf, xt, g_t, b_t, pool, n=128):
        st = pool.tile([128, 6], F32, tag="ln_st", name="ln_st")
        nc.vector.bn_stats(st[0:n, :], xt)
        mv = pool.tile([128, 2], F32, tag="ln_mv", name="ln_mv")
        nc.vector.bn_aggr(mv[0:n, :], st[0:n, :])
        sq = pool.tile([128, 1], F32, tag="ln_sq", name="ln_sq")
        nc.scalar.activation(sq[0:n, :], mv[0:n, 1:2], AF.Sqrt, bias=eps_s[0:n, :], scale=1.0)
        r = pool.tile([128, 1], F32, tag="ln_r", name="ln_r")
        nc.vector.reciprocal(r[0:n, :], sq[0:n, :])
        mr = pool.tile([128, 1], F32, tag="ln_mr", name="ln_mr")
        nc.vector.tensor_mul(mr[0:n, :], mv[0:n, 0:1], r[0:n, :])
        nmr = pool.tile([128, 1], F32, tag="ln_nmr", name="ln_nmr")
        nc.vector.tensor_scalar_mul(nmr[0:n, :], mr[0:n, :], -1.0)
        xn = pool.tile([128, D], F32, tag="ln_xn", name="ln_xn")
        nc.scalar.activation(xn[0:n, :], xt, AF.Identity, bias=nmr[0:n, :], scale=r[0:n, :])
        nc.vector.tensor_mul(dst_f32, xn[0:n, :], g_t[0:n, :])
        nc.vector.tensor_add(dst_f32, dst_f32, b_t[0:n, :])
        nc.vector.tensor_copy(dst_bf, dst_f32)

    def transpose_to(dst_bf, src_tiles, n, ncol=128):
        ps = pp_tp.tile([128, T * 128], BF16, tag="tp", name="tp")
        for i in range(n):
            nc.tensor.transpose(ps[:, i * ncol:(i + 1) * ncol], src_tiles[i],
                                ident[0:ncol, 0:ncol])
        nc.scalar.copy(dst_bf[:, 0:n * ncol], ps[:, 0:n * ncol])

    def project_T(dst, W_bf, xT_full, n_l, scale=None, c0=0, src_c0=None):
        if src_c0 is None:
            src_c0 = c0
        for jm in range(2):
            for q4 in range(0, n_l, 256):
                w = min(256, n_l - q4)
                ps = pp_s.tile([128, 288], F32, tag="proj", name="proj")
                for Dj in range(2):
                    nc.tensor.matmul(
                        ps[:, 0:w],
                        W_bf[:, Dj, jm * 128:(jm + 1) * 128],
                        xT_full[:, Dj, src_c0 + q4:src_c0 + q4 + w],
                        start=(Dj == 0), stop=(Dj == 1))
                if scale is None:
                    nc.scalar.copy(dst[:, jm, c0 + q4:c0 + q4 + w], ps[:, 0:w])
                else:
                    nc.scalar.mul(dst[:, jm, c0 + q4:c0 + q4 + w], ps[:, 0:w], scale)

    def project_V(dst, W_bf, xT_full, tiles, src_base=0):
        for ti, t in enumerate(tiles):
            for dh in range(0, D, 256):
                ps = pp_s.tile([128, 288], F32, tag="proj", name="proj")
                for Dj in range(2):
                    nc.tensor.matmul(
                        ps[:, 0:256],
                        xT_full[:, Dj, src_base + ti * 128:src_base + (ti + 1) * 128],
                        W_bf[:, Dj, dh:dh + 256],
                        start=(Dj == 0), stop=(Dj == 1))
                nc.scalar.copy(dst[:, t, dh:dh + 256], ps[:, 0:256])

    # ================= pair block =================
    def pair_block(b):
        p_nat = p_nats[b]
        flat = p_nat.rearrange("p q j c -> p (q j) c")
        t_sq = sq_pool.tile([128, QBLK * T, C // 2], BF16, tag="t_sq", name="t_sq")
        nc.vector._custom_dve(SQ2, out=t_sq, in0=flat[:, :, 0:C // 2],
                              in1=flat[:, :, C // 2:C])
        u_sq = sq_pool.tile([128, QBLK * T, C // 4], BF16, tag="u_sq", name="u_sq")
        nc.vector.tensor_add(u_sq, t_sq[:, :, 0:C // 4], t_sq[:, :, C // 4:C // 2])
        sumsq = st_pool.tile([128, QBLK * T], F32, tag="sumsq", name="sumsq")
        nc.vector.tensor_reduce(out=sumsq, in_=u_sq, axis=AX.X, op=OP.add)

        sums = st_pool.tile([128, QBLK * T], F32, tag="sums", name="sums")
        ps_S_list = []
        for hb in range(2):
            ps_S = pp_s.tile([128, 288], F32, tag="proj", name="ps_S")
            ps_S_list.append(ps_S)
            for qi in range(4):
                qq = hb * 4 + qi
                ps_t = pp_tp.tile([128, T * 128], BF16, tag="tp", name="tp")
                for j in range(T):
                    nc.tensor.transpose(ps_t[:, j * 128:(j + 1) * 128],
                                        p_nat[:, qq, j, :], ident)
                pT = pT_pool.tile([128, T * 128], BF16, tag="pT", name="pT")
                if qq == 3:
                    nc.vector.tensor_copy(pT, ps_t)  # offload one copy to DVE
                else:
                    nc.scalar.copy(pT, ps_t)
                for j in range(T):
                    nc.tensor.matmul(
                        ps_S[:, (qi * T + j) * 9:(qi * T + j) * 9 + 9],
                        pT[:, j * 128:(j + 1) * 128], Wstat,
                        start=True, stop=True)
            nc.vector.tensor_copy(
                sums[:, hb * 32:(hb + 1) * 32],
                bass.AP(tensor=ps_S.tensor, offset=ps_S.offset + 8,
                        ap=[ps_S.ap[0], [9, 32]]))

        t1 = st_pool.tile([128, QBLK * T], F32, tag="t1", name="t1")
        nc.vector.tensor_mul(t1, sums, sums)
        v128 = st_pool.tile([128, QBLK * T], F32, tag="v128", name="v128")
        nc.vector.tensor_scalar_mul(v128, sumsq, float(C))
        nc.vector.tensor_sub(v128, v128, t1)
        sqv = st_pool.tile([128, QBLK * T], F32, tag="sqv", name="sqv")
        nc.scalar.activation(sqv, v128, AF.Sqrt, bias=eps_t, scale=1.0)
        r_all = st_pool.tile([128, QBLK * T], F32, tag="r_all", name="r_all")
        nc.vector.reciprocal(r_all, sqv)

        # assembly into the right half: S_h[p, j*HALF*H + q'*H + h] * r
        S_dst = S_half[b // (NBLK // 2)]
        qbase = (b % (NBLK // 2)) * QBLK
        for hb in range(2):
            ps_S = ps_S_list[hb]
            out_ap = bass.AP(
                tensor=S_dst.tensor,
                offset=S_dst.offset + (qbase + hb * 4) * H,
                ap=[S_dst.ap[0], [H, 4], [HALF * H, T], [1, H]])
            in_ap = bass.AP(
                tensor=ps_S.tensor, offset=ps_S.offset,
                ap=[ps_S.ap[0], [9 * T, 4], [9, T], [1, H]])
            r_ap = bass.AP(
                tensor=r_all.tensor, offset=r_all.offset + hb * 4 * T,
                ap=[r_all.ap[0], [T, 4], [1, T], [0, H]])
            nc.vector.tensor_tensor(out=out_ap, in0=in_ap, in1=r_ap, op=OP.mult)

    # ================= attention / gate (per 64-q half) =================
    def attn_head(h, h8, KT, QT, V, S_bias, w_t, o_half):
        """One head of k-part attention for half h; o_half [64, D] bf16."""
        h0 = h * HALF
        ps_o = pp_l.tile([128, 128], F32, tag="ps_o", name="ps_o", bufs=1)
        E = roll3.tile([128, T * HALF], BF16, tag="E", name="E")
        ps_lg = pp_l.tile([128, T * HALF], F32, tag="ps_lg", name="ps_lg")
        jh, rh = h8 // 4, (h8 % 4) * 32
        for j in range(T):
            nc.tensor.matmul(
                ps_lg[:, j * HALF:(j + 1) * HALF],
                KT[rh:rh + 32, jh, j * 128:(j + 1) * 128],
                QT[rh:rh + 32, jh, h0:h0 + HALF],
                start=True, stop=(S_bias is None),
                tile_position=(rh, 0))
        if S_bias is not None:
            # one accumulating bias add over all (j, q') via identity matmul
            bias_ap = bass.AP(
                tensor=S_bias.tensor,
                offset=S_bias.offset + h8,
                ap=[S_bias.ap[0], [HALF * H, T], [H, HALF]])
            nc.tensor.matmul(ps_lg, ident, bias_ap, start=False, stop=True)
        if w_t is not None:
            nc.scalar.activation(E, ps_lg, AF.Exp, bias=w_t[:, h8:h8 + 1], scale=1.0)
        else:
            nc.scalar.activation(E, ps_lg, AF.Exp, bias=0.0, scale=1.0)
        for j in range(T):
            nc.tensor.matmul(ps_o[0:HALF, 0:DH], E[:, j * HALF:(j + 1) * HALF],
                             V[:, j, h8 * DH:(h8 + 1) * DH],
                             start=(j == 0), stop=False)
            nc.tensor.matmul(ps_o[0:HALF, DH:DH + 1], E[:, j * HALF:(j + 1) * HALF],
                             ones_k, start=(j == 0), stop=(j == T - 1))
        recip = roll3.tile([128, 1], F32, tag="recip", name="recip")
        nc.vector.reciprocal(recip[0:HALF, :], ps_o[0:HALF, DH:DH + 1])
        nc.vector.tensor_scalar_mul(o_half[0:HALF, h8 * DH:(h8 + 1) * DH],
                                    ps_o[0:HALF, 0:DH], recip[0:HALF, :])

    def gate_proj_residual_half(h0, xT_my, Wg_bf, bg_row, Wo_bf, bo_row, o_half,
                                x_prev_half, x_new_f, x_new_bf, hpool):
        ps_g = pp_s.tile([128, 288], F32, tag="proj", name="proj")
        for Dj in range(2):
            nc.tensor.matmul(ps_g[0:HALF, 0:256], xT_my[:, Dj, h0:h0 + HALF],
                             Wg_bf[:, Dj, :], start=(Dj == 0), stop=False)
        nc.tensor.matmul(ps_g[0:HALF, 0:256], ones_col[:, 0:HALF], bg_row,
                         start=False, stop=True)
        g_sb = hpool.tile([128, D], BF16, tag="g_sb", name="g_sb")
        nc.scalar.activation(g_sb[0:HALF, :], ps_g[0:HALF, 0:256], AF.Sigmoid,
                             bias=0.0, scale=1.0)
        go = hpool.tile([128, D], BF16, tag="go", name="go")
        nc.vector.tensor_mul(go[0:HALF, :], g_sb[0:HALF, :], o_half[0:HALF, :])
        goT = hpool.tile([128, 2, HALF], BF16, tag="goT", name="goT")
        for jm in range(2):
            transpose_to(goT[:, jm, :], [go[0:HALF, jm * 128:(jm + 1) * 128]], 1,
                         ncol=HALF)
        ps_y = pp_s.tile([128, 288], F32, tag="proj", name="proj")
        for Dj in range(2):
            nc.tensor.matmul(ps_y[0:HALF, 0:256], goT[:, Dj, 0:HALF],
                             Wo_bf[:, Dj, :], start=(Dj == 0), stop=False)
        nc.tensor.matmul(ps_y[0:HALF, 0:256], ones_col[:, 0:HALF], bo_row,
                         start=False, stop=True)
        nc.vector.tensor_add(x_new_f[0:HALF, :], x_prev_half[0:HALF, :],
                             ps_y[0:HALF, 0:256])
        nc.vector.tensor_copy(x_new_bf[0:HALF, :], x_new_f[0:HALF, :])

    def row_gate_and_gather(h):
        h0 = h * HALF
        gate_proj_residual_half(h0, x0Tmy, weights["row_Wg"], bias_rows["row_bg"],
                                weights["row_Wo"], bias_rows["row_bo"], o_row[h],
                                x0my_f[h], x1_f[h], x1_bf[h], roll3)
        for jm in range(2):
            transpose_to(x1Tmy[:, jm, h0:h0 + HALF],
                         [x1_bf[h][0:HALF, jm * 128:(jm + 1) * 128]], 1, ncol=HALF)
        for jm in range(2):
            nc.sync.dma_start(out=gin[h].ap()[jm * 128:(jm + 1) * 128, :],
                              in_=x1Tmy[:, jm, h0:h0 + HALF])
        nc.gpsimd.collective_compute(
            "AllGather", OP.bypass,
            replica_groups=[list(range(NCORES))],
            ins=[gin[h].ap().opt()],
            outs=[gout[h].ap().opt()])

    def col_kv_for_half(h):
        # read back AG#h -> x1T[h] cols (i, q'), then project K/V tiles
        gout_r = gout[h].ap().rearrange("(i a p) q -> p a i q", i=NCORES, a=2)
        x1T_4d = x1T[h].rearrange("p a (i q) -> p a i q", i=NCORES)
        for jm in range(2):
            nc.sync.dma_start(out=x1T_4d[:, jm, :, :], in_=gout_r[:, jm, :, :])
        project_T(KT_col, weights["col_Wk"], x1T[h], 512, c0=h * 512, src_c0=0)
        project_V(V_col, weights["col_Wv"], x1T[h], range(4 * h, 4 * h + 4))

    # ================= interleaved schedule =================
    def setup_step(b):
        if b == 0:
            for j in range(T):
                layer_norm(x0_f[:, j, :], x0_bf[:, j, :], msa_g_t[:, j, :],
                           G_node, B_node, roll)
            for h in range(2):
                layer_norm(x0my_f[h][0:HALF, :], x0my_bf[h][0:HALF, :],
                           msa_my_t[h][0:HALF, :], G_node, B_node, roll, n=HALF)
        elif b == 1:
            wbf("row_Wk")
            wbf("row_Wq")
            for jm in range(2):
                transpose_to(x0T[:, jm, :],
                             [x0_bf[:, j, jm * 128:(jm + 1) * 128] for j in range(T)], T)
            for h in range(2):
                for jm in range(2):
                    transpose_to(x0Tmy[:, jm, h * HALF:(h + 1) * HALF],
                                 [x0my_bf[h][0:HALF, jm * 128:(jm + 1) * 128]], 1,
                                 ncol=HALF)
        elif b == 2:
            project_T(KT_row, weights["row_Wk"], x0T, L)
            project_T(QT_row, weights["row_Wq"], x0Tmy, 128, scale=SCALE)
        elif b == 3:
            wbf("row_Wv")
            project_V(V_row, weights["row_Wv"], x0T, range(T))
        elif b == 4:
            wbf("row_Wg")
            wbf("row_Wo")
            load_bias("row_bg")
            load_bias("row_bo")
        elif b == 5:
            wbf("col_Wk")
            wbf("col_Wv")
        elif b == 6:
            wbf("col_Wq")
            wbf("col_Wg")
            wbf("col_Wo")
            load_bias("col_bg")
            load_bias("col_bo")
        elif b == 7:
            wbf("ff_W1")
            wbf("ff_W2")
            load_bias("ff_b2")
            b1T = P([128, 2], F32, tag="b1T")
            nc.sync.dma_start(out=b1T, in_=v["ff_b1"].rearrange("(a p) o -> p (a o)", p=128))
            bias_rows["ff_b1T"] = b1T
            g = _bcast(nc, pers, v["ff_ln_g"], tag="G_ff")
            bias_rows["G_ff"] = g
            bb = _bcast(nc, pers, v["ff_ln_b"], tag="B_ff")
            bias_rows["B_ff"] = bb
        elif 8 <= b <= 11:
            for h8 in range(2 * (b - 8), 2 * (b - 8) + 2):
                attn_head(0, h8, KT_row, QT_row, V_row, S_half[0], w_tile, o_row[0])
        elif b == 12:
            row_gate_and_gather(0)
        elif b == 14:
            col_kv_for_half(0)

    for b in range(NBLK):
        if b + 3 < NBLK:
            issue_pair_dma(b + 3)
        pair_block(b)
        setup_step(b)

    # ---- row attention half 1 + AG#2 ----
    for h8 in range(H):
        attn_head(1, h8, KT_row, QT_row, V_row, S_half[1], w_tile, o_row[1])
    row_gate_and_gather(1)
    col_kv_for_half(1)

    # ---- col attention + FF per half ----
    QT_col = P([128, 2, 128], BF16, tag="QT_col")
    project_T(QT_col, weights["col_Wq"], x1Tmy, 128, scale=SCALE)

    for h in range(2):
        h0 = h * HALF
        o_col = roll3.tile([128, D], BF16, tag="o_col", name="o_col")
        for h8 in range(H):
            attn_head(h, h8, KT_col, QT_col, V_col, None, None, o_col)
        x2_f = roll3.tile([128, D], F32, tag="x2_f", name="x2_f")
        x2_bf = roll3.tile([128, D], BF16, tag="x2_bf", name="x2_bf")
        gate_proj_residual_half(h0, x1Tmy, weights["col_Wg"], bias_rows["col_bg"],
                                weights["col_Wo"], bias_rows["col_bo"], o_col,
                                x1_f[h], x2_f, x2_bf, roll3)

        h_f = roll3.tile([128, D], F32, tag="h_f", name="h_f")
        h_bf = roll3.tile([128, D], BF16, tag="h_bf", name="h_bf")
        layer_norm(h_f[0:HALF, :], h_bf[0:HALF, :], x2_f[0:HALF, :],
                   bias_rows["G_ff"], bias_rows["B_ff"], roll3, n=HALF)
        hT = roll3.tile([128, 2, HALF], BF16, tag="hT", name="hT")
        for jm in range(2):
            transpose_to(hT[:, jm, :], [h_bf[0:HALF, jm * 128:(jm + 1) * 128]], 1,
                         ncol=HALF)
        a1T = roll3.tile([128, 2, HALF], BF16, tag="a1T", name="a1T")
        for jm in range(2):
            ps_z = pp_s.tile([128, 288], F32, tag="proj", name="proj")
            for Dj in range(2):
                nc.tensor.matmul(ps_z[:, 0:HALF],
                                 weights["ff_W1"][:, Dj, jm * 128:(jm + 1) * 128],
                                 hT[:, Dj, :], start=(Dj == 0), stop=(Dj == 1))
            nc.scalar.activation(a1T[:, jm, :], ps_z[:, 0:HALF], AF.Relu,
                                 bias=bias_rows["ff_b1T"][:, jm:jm + 1], scale=1.0)
        ps_y = pp_s.tile([128, 288], F32, tag="proj", name="proj")
        for jm in range(2):
            nc.tensor.matmul(ps_y[0:HALF, 0:256], a1T[:, jm, :],
                             weights["ff_W2"][:, jm, :],
                             start=(jm == 0), stop=False)
        nc.tensor.matmul(ps_y[0:HALF, 0:256], ones_col[:, 0:HALF],
                         bias_rows["ff_b2"], start=False, stop=True)
        out_sb = roll3.tile([128, D], F32, tag="out_sb", name="out_sb")
        nc.vector.tensor_add(out_sb[0:HALF, :], x2_f[0:HALF, :], ps_y[0:HALF, 0:256])
        nc.sync.dma_start(out=out_my[h0:h0 + HALF, :], in_=out_sb[0:HALF, :])
    ctx.close()


_NC_CACHE = None


def make_in_maps(common, msa, pair):
    in_maps = []
    for i in range(NCORES):
        m = dict(common)
        m["msa_my"] = np.ascontiguousarray(msa[i * MYQ:(i + 1) * MYQ, :])
        m["pair_my"] = np.ascontiguousarray(pair[i * MYQ:(i + 1) * MYQ, :, :])
        in_maps.append(m)
    return in_maps


def kernel(**inputs):
    global _NC_CACHE
    if _NC_CACHE is None:
        _NC_CACHE = build()
    nc = _NC_CACHE

    msa = np.asarray(inputs["msa"]).reshape(L, D).astype(np.float32)
    pair = np.asarray(inputs["pair"]).reshape(L, L, C).astype(np.float32)

    def f(name, shape):
        return np.ascontiguousarray(
            np.asarray(inputs[name]).reshape(shape).astype(np.float32))

    common = {
        "msa": msa,
        "ln_node_g": f("ln_node_g", (1, D)), "ln_node_b": f("ln_node_b", (1, D)),
        "ln_pair_g": f("ln_pair_g", (C, 1)), "ln_pair_b": f("ln_pair_b", (C, 1)),
        "row_Wq": f("row_Wq", (D, D)), "row_Wk": f("row_Wk", (D, D)),
        "row_Wv": f("row_Wv", (D, D)), "row_Wb": f("row_Wb", (C, H)),
        "row_Wg": f("row_Wg", (D, D)), "row_bg": f("row_bg", (1, D)),
        "row_Wo": f("row_Wo", (D, D)), "row_bo": f("row_bo", (1, D)),
        "col_Wq": f("col_Wq", (D, D)), "col_Wk": f("col_Wk", (D, D)),
        "col_Wv": f("col_Wv", (D, D)),
        "col_Wg": f("col_Wg", (D, D)), "col_bg": f("col_bg", (1, D)),
        "col_Wo": f("col_Wo", (D, D)), "col_bo": f("col_bo", (1, D)),
        "ff_ln_g": f("ff_ln_g", (1, D)), "ff_ln_b": f("ff_ln_b", (1, D)),
        "ff_W1": f("ff_W1", (D, D)), "ff_b1": f("ff_b1", (D, 1)),
        "ff_W2": f("ff_W2", (D, D)), "ff_b2": f("ff_b2", (1, D)),
    }
    in_maps = make_in_maps(common, msa, pair)
    res = run_bass_kernel_spmd(nc, in_maps, core_ids=list(range(NCORES)))
    out = np.concatenate([res.results[i]["out_my"] for i in range(NCORES)], axis=0)
    return out.reshape(1, L, D).astype(np.float32)


if __name__ == "__main__":
    build()
    print("build OK")
